# revision 1
# baseline (speedup 1.0000x reference)
"""Autoregressive LSTM cell (B=256, T=256, D=256, H=1024, O=256) on 8 TRN2 cores.

Strategy: pure data-parallel over batch (32 rows/core, no collectives).
Per step t (sequential, 256 steps):
    z = x_t @ Wxx + y_{t-1} @ Wxy + h_{t-1} @ Wh     (+b)
    i,f,g,o gates -> c = sig(f)*c + sig(i)*tanh(g); h = sig(o)*tanh(c)
    y = tanh(h @ Wd + bd)
Matmuls are "activation-stationary": lhsT = activation^T [K<=128, M=32batch],
rhs streams bf16 weight columns at 1 column/cycle (fp32 PSUM accumulation;
fp32 gate math keeps the recurrence error ~1e-2-bounded).
4-way PE column tiling (tile_position col groups) packs 4 batch-32 matmuls
concurrently, writing z in a stacked PSUM layout:
  z_ps [128, 1024]: position (32j+b, 512*beta + n) = z_perm[2048*beta + 512j + n]
Weight columns are host-permuted so that
  bank0 (cols 0:512)  = [ sig-gate i | sig-gate f ] column-paired per channel
  bank1 (cols 512:1024) = [ tanh-gate g | sig-gate o ]
with channel(p=32j+b, n) = 256j + (n mod 256): all gate elementwise ops are
partition-aligned and the c update is a free-dim-shifted add.
h/y are fed back transposed via PE transpose-mode matmuls.

Overlap structure: the x-part matmuls of step t+1 are software-pipelined into
step t (the in-order PE would otherwise stall on the gate chain), z-chunks are
ordered x->h->y so the y-feedback chain hides under the h-matmul span, and the
gate chain runs in two column halves so each half's transpose/hT-copy releases
the even/odd Wh chunks of the next step independently.

Measured (axon, differential timing T=256 vs T=8 over 40 reps): ~1.3-1.8 ms
device time for the full 256-step recurrence (~5-7 us/step), vs ~1.37 ms
8-core bf16 compute roofline. End-to-end error vs the fp32 jax reference:
scale-relative absmax 7.8e-3.
"""

import sys

for p in ("/opt/trn_rl_repo",):
    if p not in sys.path:
        sys.path.insert(0, p)

from contextlib import ExitStack

import numpy as np

import concourse.bacc as bacc
import concourse.bass as bass
import concourse.mybir as mybir
import concourse.tile as tile
from concourse.masks import make_identity

F32 = mybir.dt.float32
AF = mybir.ActivationFunctionType

B, T, D, H, O = 256, 256, 256, 1024, 256
NCORES = 8
BL = B // NCORES  # 32
G4 = 4 * H  # 4096
KX, KY, KH = D // 128, O // 128, H // 128  # 2, 2, 8


def gate_perm() -> np.ndarray:
    """Map stored z column position -> original gate column (i,f,g,o order)."""
    perm = np.empty(G4, dtype=np.int64)
    for beta in (0, 1):
        for j in range(4):
            for half in (0, 1):
                gate = (0, 1, 2, 3)[2 * beta + half]
                src = 1024 * gate + 256 * j
                pos = 2048 * beta + 512 * j + 256 * half
                perm[pos : pos + 256] = np.arange(src, src + 256)
    return perm


def _hT_off(c: int) -> int:
    """Column offset of h^T chunk c (channels 128c:128c+128) inside hT_sb."""
    return 128 * (c % 2) + 32 * (c // 2)


def round_f32r(a: np.ndarray) -> np.ndarray:
    """Round-to-nearest-even fp32 -> fp32r (low 12 mantissa bits zeroed)."""
    u = np.ascontiguousarray(a, dtype=np.float32).view(np.uint32)
    lsb = (u >> np.uint32(12)) & np.uint32(1)
    u = (u + np.uint32(0x7FF) + lsb) & np.uint32(0xFFFFF000)
    return u.view(np.float32)


FUNNEL = False


def build_nc(T_steps: int = T, use_bias_z: bool = False, use_bias_y: bool = False,
             mm_dt=mybir.dt.bfloat16):
    nc = bacc.Bacc()

    xT_d = nc.declare_dram_parameter("xT", [T_steps, 128, 2 * BL], mm_dt,
                                     isOutput=False)
    Wxx_d = nc.declare_dram_parameter("Wxx", [D, G4], mm_dt, isOutput=False)
    Wxy_d = nc.declare_dram_parameter("Wxy", [O, G4], mm_dt, isOutput=False)
    Wh_d = nc.declare_dram_parameter("Wh", [H, G4], mm_dt, isOutput=False)
    Wd_d = nc.declare_dram_parameter("Wd", [H, O], mm_dt, isOutput=False)
    bz_d = by_d = None
    if use_bias_z:
        bz_d = nc.declare_dram_parameter("bz", [128, 1024], F32, isOutput=False)
    if use_bias_y:
        by_d = nc.declare_dram_parameter("by", [BL, O], F32, isOutput=False)
    ys_d = nc.declare_dram_parameter("ys", [BL, T_steps, O], F32, isOutput=True)

    def mc(ap):
        return ap.bitcast(mm_dt) if ap.dtype != mm_dt else ap

    with tile.TileContext(nc) as tc:
        with ExitStack() as ctx:
            wpool = ctx.enter_context(tc.tile_pool(name="weights", bufs=1))
            state = ctx.enter_context(tc.tile_pool(name="state", bufs=1))
            xpool = ctx.enter_context(tc.tile_pool(name="xin", bufs=3))
            gpool = ctx.enter_context(tc.tile_pool(name="gates", bufs=1))
            hpool = ctx.enter_context(tc.tile_pool(name="hT", bufs=1))
            ypool = ctx.enter_context(tc.tile_pool(name="yt", bufs=1))
            zpsum = ctx.enter_context(tc.tile_pool(name="zps", bufs=2, space="PSUM"))
            ypsum = ctx.enter_context(tc.tile_pool(name="yps", bufs=2, space="PSUM"))
            tpsum = ctx.enter_context(tc.tile_pool(name="tps", bufs=2, space="PSUM"))

            Wxx_sb = wpool.tile([128, KX * G4], mm_dt)
            Wxy_sb = wpool.tile([128, KY * G4], mm_dt)
            Wh_sb = wpool.tile([128, KH * G4], mm_dt)
            Wd_sb = wpool.tile([128, KH * O], mm_dt)
            # Matmult instructions can carry at most ONE sem wait in this
            # lowering; every matmul dependency must resolve to a single DVE
            # sem value. Weight DMAs are therefore "laundered" through
            # in-place DVE copies (one per DMA so each copy waits on one
            # DMA-queue sem only).
            for k in range(KX):
                nc.sync.dma_start(Wxx_sb[:, k * G4 : (k + 1) * G4],
                                  Wxx_d[k * 128 : (k + 1) * 128, :])
                nc.vector.tensor_copy(Wxx_sb[:, k * G4 : (k + 1) * G4],
                                      Wxx_sb[:, k * G4 : (k + 1) * G4])
            for k in range(KY):
                nc.sync.dma_start(Wxy_sb[:, k * G4 : (k + 1) * G4],
                                  Wxy_d[k * 128 : (k + 1) * 128, :])
                nc.vector.tensor_copy(Wxy_sb[:, k * G4 : (k + 1) * G4],
                                      Wxy_sb[:, k * G4 : (k + 1) * G4])
            for k in range(KH):
                nc.sync.dma_start(Wh_sb[:, k * G4 : (k + 1) * G4],
                                  Wh_d[k * 128 : (k + 1) * 128, :])
                nc.vector.tensor_copy(Wh_sb[:, k * G4 : (k + 1) * G4],
                                      Wh_sb[:, k * G4 : (k + 1) * G4])
                nc.sync.dma_start(Wd_sb[:, k * O : (k + 1) * O],
                                  Wd_d[k * 128 : (k + 1) * 128, :])
                nc.vector.tensor_copy(Wd_sb[:, k * O : (k + 1) * O],
                                      Wd_sb[:, k * O : (k + 1) * O])
            if use_bias_z:
                bz_sb = wpool.tile([128, 1024], F32)
                nc.sync.dma_start(bz_sb[:], bz_d[:, :])
            if use_bias_y:
                by_sb = wpool.tile([BL, O], F32)
                nc.sync.dma_start(by_sb[:], by_d[:, :])

            # identity for PE transposes (bf16: f32 transpose-mode faults on
            # hw); I64 in both partition halves so the fmap can start at
            # partition 0 or 64 (must match the weights)
            ident = wpool.tile([128, 128], mm_dt)
            make_identity(nc, ident[:])
            nc.vector.tensor_copy(ident[:], ident[:])  # launder Pool dep -> DVE

            # c state, channel(32j+b, n) = 256j + n
            c_sb = state.tile([128, 256], F32)
            nc.gpsimd.memset(c_sb[:], 0.0)

            def emit_z_mms(z_tile, chunks, start, stop):
                nck = len(chunks)
                for ci, (lhsT, wtile, coff) in enumerate(chunks):
                    for beta in range(2):
                        for j in range(4):
                            w_lo = coff + 2048 * beta + 512 * j
                            nc.tensor.matmul(
                                z_tile[32 * j : 32 * (j + 1),
                                       512 * beta : 512 * (beta + 1)],
                                mc(lhsT),
                                mc(wtile[:, w_lo : w_lo + 512]),
                                start=(start and ci == 0),
                                stop=(stop and ci == nck - 1),
                                tile_position=(0, 32 * j),
                                skip_group_check=True,
                            )

            def load_x(t):
                xT_sb = xpool.tile([128, 2 * BL], mm_dt, name="xT_sb")
                nc.sync.dma_start(xT_sb[:], xT_d[t])
                # launder the x DMA-queue sem into the DVE sem
                xr_sb = xpool.tile([128, 2 * BL], mm_dt, name="xr_sb")
                nc.vector.tensor_copy(xr_sb[:], xT_sb[:])
                return [(xr_sb[:, bass.ts(k, BL)], Wxx_sb, k * G4)
                        for k in range(KX)]

            hT_prev = None
            yT_prev = None
            # software pipeline: the x-part of step t+1 is issued during step
            # t, so the in-order PE has independent work while the gate chain
            # (ACT/DVE) of step t runs.
            z_ps = zpsum.tile([128, 1024], F32, name="z_ps")
            emit_z_mms(z_ps, load_x(0), start=True, stop=(T_steps == 1))
            for t in range(T_steps):
                if t > 0:
                    # h first, y last: the y feedback chain (Wd+tanh+cast+
                    # transpose) of step t-1 gets the h-matmul span as slack
                    chunks = [(hT_prev[:, _hT_off(k) : _hT_off(k) + BL], Wh_sb,
                               k * G4) for k in (0, 2, 4, 6, 1, 3, 5, 7)]
                    chunks += [(yT_prev[:, bass.ts(k, BL)], Wxy_sb, k * G4)
                               for k in range(KY)]
                    emit_z_mms(z_ps, chunks, start=False, stop=True)
                if t + 1 < T_steps:
                    z_next = zpsum.tile([128, 1024], F32, name="z_ps")
                    emit_z_mms(z_next, load_x(t + 1), start=True, stop=False)
                else:
                    z_next = None

                # gate math: <=1 PSUM operand per DVE op
                if use_bias_z:
                    nc.vector.tensor_add(z_ps[:, 0:512], z_ps[:, 0:512],
                                         bz_sb[:, 0:512])
                    nc.vector.tensor_add(z_ps[:, 512:1024], z_ps[:, 512:1024],
                                         bz_sb[:, 512:1024])
                # gate chain split into column halves: half 0 finishes ->
                # its transpose + hT copy run while half 1 still computes, so
                # the even hT-chunk matmuls of step t+1 start earlier
                tg_sb = gpool.tile([128, 256], F32, name="tg_sb")
                o_sb = gpool.tile([128, 256], F32, name="o_sb")
                h_stk = gpool.tile([128, 256], mm_dt, name="h_stk")
                tr_ps = tpsum.tile([128, 320], mm_dt, name="tr_ps")
                hT_sb = hpool.tile([128, 256], mm_dt, name="hT_sb")
                for hf in range(2):
                    s = slice(128 * hf, 128 * hf + 128)
                    nc.scalar.activation(tg_sb[:, s], z_ps[:, 512 + 128 * hf :
                                                           640 + 128 * hf],
                                         AF.Tanh)
                    nc.scalar.activation(z_ps[:, s], z_ps[:, s], AF.Sigmoid)
                    nc.vector.tensor_mul(tg_sb[:, s], z_ps[:, s], tg_sb[:, s])
                    nc.scalar.activation(z_ps[:, 256 + 128 * hf : 384 + 128 * hf],
                                         z_ps[:, 256 + 128 * hf : 384 + 128 * hf],
                                         AF.Sigmoid)
                    nc.vector.tensor_mul(c_sb[:, s],
                                         z_ps[:, 256 + 128 * hf : 384 + 128 * hf],
                                         c_sb[:, s])
                    nc.scalar.activation(o_sb[:, s], z_ps[:, 768 + 128 * hf :
                                                          896 + 128 * hf],
                                         AF.Sigmoid)
                    nc.vector.tensor_add(c_sb[:, s], tg_sb[:, s], c_sb[:, s])
                    nc.scalar.activation(tg_sb[:, s], c_sb[:, s], AF.Tanh)
                    nc.vector.tensor_mul(h_stk[:, s], o_sb[:, s], tg_sb[:, s])
                    nc.tensor.transpose(tr_ps[:, s], h_stk[:, s], ident[:])
                    nc.vector.tensor_copy(hT_sb[:, s], tr_ps[:, s])

                # y = tanh(h @ Wd + bd)
                y_ps = ypsum.tile([BL, O], F32, name="y_ps")
                if FUNNEL:
                    nc.vector.tensor_copy(y_ps[:], Wxx_sb[0:BL, 0:256])
                for k in range(KH):
                    nc.tensor.matmul(
                        y_ps[:],
                        mc(hT_sb[:, _hT_off(k) : _hT_off(k) + BL]),
                        mc(Wd_sb[:, k * O : (k + 1) * O]),
                        start=(k == 0),
                        stop=(k == KH - 1),
                    )
                # own double-buffered tile so the output DMA never blocks
                # the next step's gate ACTs
                y_sb = ypool.tile([BL, O], F32, name="y_sb", bufs=2)
                if use_bias_y:
                    nc.vector.tensor_add(y_ps[:], y_ps[:], by_sb[:])
                nc.scalar.activation(y_sb[:], y_ps[:], AF.Tanh)
                nc.sync.dma_start(ys_d[:, t, :], y_sb[:])
                # cast y for the bf16 PE-transposes (also launders ACT -> DVE)
                y_bf = ypool.tile([BL, O], mm_dt, name="y_bf")
                nc.vector.tensor_copy(y_bf[:], y_sb[:])

                # y -> yT via 2 PE transposes
                for q in range(2):
                    nc.tensor.transpose(
                        tr_ps[:, 256 + 32 * q : 256 + 32 * (q + 1)],
                        y_bf[0:BL, 128 * q : 128 * (q + 1)],
                        ident[0:32, 0:32],
                    )
                yT_sb = ypool.tile([128, 2 * BL], mm_dt, name="yT_sb")
                nc.vector.tensor_copy(yT_sb[:], tr_ps[:, 256:320])

                hT_prev = hT_sb
                yT_prev = yT_sb
                z_ps = z_next

    nc.compile()
    return nc


def prep_inputs(x, Wx, Wh, b, Wd, bd, T_steps: int = T):
    """Host-side shard + relayout. Returns (in_maps, use_bias_z, use_bias_y)."""
    x = np.asarray(x, dtype=np.float32)[:, :T_steps, :]
    Wx = np.asarray(Wx, dtype=np.float32)
    Wh = np.asarray(Wh, dtype=np.float32)
    b = np.asarray(b, dtype=np.float32)
    Wd = np.asarray(Wd, dtype=np.float32)
    bd = np.asarray(bd, dtype=np.float32)

    import ml_dtypes

    perm = gate_perm()
    Wxp = np.ascontiguousarray(Wx[:, perm]).astype(ml_dtypes.bfloat16)
    Whp = np.ascontiguousarray(Wh[:, perm]).astype(ml_dtypes.bfloat16)
    Wxx = np.ascontiguousarray(Wxp[:D])
    Wxy = np.ascontiguousarray(Wxp[D:])
    Wd = Wd.astype(ml_dtypes.bfloat16)

    use_bias_z = bool(np.any(b))
    use_bias_y = bool(np.any(bd))
    shared = {"Wxx": Wxx, "Wxy": Wxy, "Wh": Whp, "Wd": Wd}
    if use_bias_z:
        bp = b[perm]
        bz = np.empty((128, 1024), dtype=np.float32)
        for j in range(4):
            for beta in range(2):
                bz[32 * j : 32 * (j + 1), 512 * beta : 512 * (beta + 1)] = bp[
                    2048 * beta + 512 * j : 2048 * beta + 512 * j + 512][None, :]
        shared["bz"] = bz
    if use_bias_y:
        shared["by"] = np.broadcast_to(bd, (BL, O)).copy()

    in_maps = []
    for c in range(NCORES):
        xc = x[c * BL : (c + 1) * BL]                      # [BL, T, D]
        xT = xc.transpose(1, 2, 0)                         # [T, D, BL]
        xT = xT.reshape(T_steps, 2, 128, BL).transpose(0, 2, 1, 3)
        import ml_dtypes
        xT = np.ascontiguousarray(
            xT.reshape(T_steps, 128, 2 * BL)).astype(ml_dtypes.bfloat16)
        in_maps.append({"xT": xT, **shared})
    return in_maps, use_bias_z, use_bias_y


def kernel(x, Wx, Wh, b, Wd, bd):
    from concourse.bass_utils import run_bass_kernel_spmd

    in_maps, ubz, uby = prep_inputs(x, Wx, Wh, b, Wd, bd, T)
    nc = build_nc(T, ubz, uby)
    res = run_bass_kernel_spmd(nc, in_maps, list(range(NCORES)))
    ys = np.concatenate([res.results[c]["ys"] for c in range(NCORES)], axis=0)
    return ys.astype(np.float32)



# revision 3
# speedup vs baseline: 1.5071x; 1.5071x over previous
"""Autoregressive LSTM cell (B=256, T=256, D=256, H=1024, O=256) on 8 TRN2 cores.

Strategy: pure data-parallel over batch (32 rows/core). The end-to-end time is
dominated by host<->device transfer over the axon tunnel (~58MB/s h2d,
~42MB/s d2h), not device compute (~1.5ms), so the kernel minimizes shipped
bytes:
  - x is shipped fp16 (4MB/core);
  - weights are shipped SHARDED 1/8 per core (fp16, ~1.6MB/core) and
    reassembled on device with four DRAM AllGather collectives;
  - the output is shipped uint8: u = round(127*y) + 128, decoded on host as
    (u-128)/127 (2MB/core out + 2MB/core donated zero-buffer in).
The whole matmul/feedback pipeline runs fp16 (vs bf16 before), which drops
the fp32-reference error to ~9e-4 and leaves the u8 output quantization
(~4e-3) well inside the 2e-2 gate.

Per step t (sequential, 256 steps):
    z = x_t @ Wxx + y_{t-1} @ Wxy + h_{t-1} @ Wh     (+b)
    i,f,g,o gates -> c = sig(f)*c + sig(i)*tanh(g); h = sig(o)*tanh(c)
    y = tanh(h @ Wd + bd)
Matmuls are "activation-stationary": lhsT = activation^T [K<=128, M=32batch],
rhs streams fp16 weight columns at 1 column/cycle (fp32 PSUM accumulation;
fp32 gate math). 4-way PE column tiling (tile_position col groups) packs 4
batch-32 matmuls concurrently, writing z in a stacked PSUM layout:
  z_ps [128, 1024]: position (32j+b, 512*beta + n) = z_perm[2048*beta + 512j + n]
Weight columns are host-permuted so that
  bank0 (cols 0:512)  = [ sig-gate i | sig-gate f ] column-paired per channel
  bank1 (cols 512:1024) = [ tanh-gate g | sig-gate o ]
with channel(p=32j+b, n) = 256j + (n mod 256): all gate elementwise ops are
partition-aligned and the c update is a free-dim-shifted add.
h/y are fed back transposed via PE transpose-mode matmuls.

Overlap structure: the x-part matmuls of step t+1 are software-pipelined into
step t, z-chunks are ordered x->h->y so the y-feedback chain hides under the
h-matmul span, and the gate chain runs in two column halves.
"""

import sys

for p in ("/opt/trn_rl_repo",):
    if p not in sys.path:
        sys.path.insert(0, p)

from contextlib import ExitStack

import numpy as np

import concourse.bacc as bacc
import concourse.bass as bass
import concourse.mybir as mybir
import concourse.tile as tile
from concourse.masks import make_identity

F32 = mybir.dt.float32
U8 = mybir.dt.uint8
AF = mybir.ActivationFunctionType
ALU = mybir.AluOpType

B, T, D, H, O = 256, 256, 256, 1024, 256
NCORES = 8
BL = B // NCORES  # 32
G4 = 4 * H  # 4096
KX, KY, KH = D // 128, O // 128, H // 128  # 2, 2, 8
WXS = D // NCORES  # 32 weight-shard rows per core for Wxx/Wxy
WHS = H // NCORES  # 128 shard rows for Wh/Wd

# u8 output encoding: u = convert_u8(127*y + U8_BIAS), decoded (u-128)/127.
# The DVE float->u8 convert rounds-to-nearest (measured on hw: mean code
# offset -0.001, std 0.289), so the bias is exactly 128.0.
U8_BIAS_DEFAULT = 128.0


def gate_perm() -> np.ndarray:
    """Map stored z column position -> original gate column (i,f,g,o order)."""
    perm = np.empty(G4, dtype=np.int64)
    for beta in (0, 1):
        for j in range(4):
            for half in (0, 1):
                gate = (0, 1, 2, 3)[2 * beta + half]
                src = 1024 * gate + 256 * j
                pos = 2048 * beta + 512 * j + 256 * half
                perm[pos : pos + 256] = np.arange(src, src + 256)
    return perm


def _hT_off(c: int) -> int:
    """Column offset of h^T chunk c (channels 128c:128c+128) inside hT_sb."""
    return 128 * (c % 2) + 32 * (c // 2)


def build_nc(T_steps: int = T, use_bias_z: bool = False, use_bias_y: bool = False,
             mm_dt=mybir.dt.float16, out_u8: bool = True,
             u8_bias: float = U8_BIAS_DEFAULT):
    nc = bacc.Bacc()

    xT_d = nc.declare_dram_parameter("xT", [T_steps, 128, 2 * BL], mm_dt,
                                     isOutput=False)
    # weights arrive as row-shards: core c holds rows [c*WXS:(c+1)*WXS] of
    # Wxx/Wxy and rows [c*WHS:(c+1)*WHS] of Wh/Wd; AllGather rebuilds the
    # full matrices in DRAM (saves 7/8 of the weight bytes over the tunnel).
    Wxx_d = nc.declare_dram_parameter("Wxx", [WXS, G4], mm_dt, isOutput=False)
    Wxy_d = nc.declare_dram_parameter("Wxy", [WXS, G4], mm_dt, isOutput=False)
    Wh_d = nc.declare_dram_parameter("Wh", [WHS, G4], mm_dt, isOutput=False)
    Wd_d = nc.declare_dram_parameter("Wd", [WHS, O], mm_dt, isOutput=False)
    bz_d = by_d = None
    if use_bias_z:
        bz_d = nc.declare_dram_parameter("bz", [128, 1024], F32, isOutput=False)
    if use_bias_y:
        by_d = nc.declare_dram_parameter("by", [BL, O], F32, isOutput=False)
    out_dt = U8 if out_u8 else mm_dt
    ys_d = nc.declare_dram_parameter("ys", [BL, T_steps, O], out_dt,
                                     isOutput=True)

    def mc(ap):
        return ap.bitcast(mm_dt) if ap.dtype != mm_dt else ap

    with tile.TileContext(nc) as tc:
        with ExitStack() as ctx:
            dpool = ctx.enter_context(
                tc.tile_pool(name="dramw", bufs=1, space="DRAM"))
            wpool = ctx.enter_context(tc.tile_pool(name="weights", bufs=1))
            state = ctx.enter_context(tc.tile_pool(name="state", bufs=1))
            xpool = ctx.enter_context(tc.tile_pool(name="xin", bufs=3))
            gpool = ctx.enter_context(tc.tile_pool(name="gates", bufs=1))
            hpool = ctx.enter_context(tc.tile_pool(name="hT", bufs=1))
            ypool = ctx.enter_context(tc.tile_pool(name="yt", bufs=1))
            zpsum = ctx.enter_context(tc.tile_pool(name="zps", bufs=2, space="PSUM"))
            ypsum = ctx.enter_context(tc.tile_pool(name="yps", bufs=2, space="PSUM"))
            tpsum = ctx.enter_context(tc.tile_pool(name="tps", bufs=2, space="PSUM"))

            # c state, channel(32j+b, n) = 256j + n  (memset first: needed at
            # t=0, and it shares the gpsimd queue with the gathers below)
            c_sb = state.tile([128, 256], F32)
            nc.gpsimd.memset(c_sb[:], 0.0)

            # ---- on-device weight reassembly: DRAM AllGather per matrix ----
            # bounce buffers: collectives can't operate on I/O tensors.
            wxx_bi = dpool.tile([WXS, G4], mm_dt, name="wxx_bi")
            wxy_bi = dpool.tile([WXS, G4], mm_dt, name="wxy_bi")
            wh_bi = dpool.tile([WHS, G4], mm_dt, name="wh_bi")
            wd_bi = dpool.tile([WHS, O], mm_dt, name="wd_bi")
            Wxx_g = dpool.tile([D, G4], mm_dt, name="Wxx_g")
            Wxy_g = dpool.tile([O, G4], mm_dt, name="Wxy_g")
            Wh_g = dpool.tile([H, G4], mm_dt, name="Wh_g")
            Wd_g = dpool.tile([H, O], mm_dt, name="Wd_g")
            RG = [list(range(NCORES))]
            # gather order = first-use order: Wxx (t=0 z), Wd (t=0 y), Wh/Wxy
            # (t=1 z)
            for src, bi, g in ((Wxx_d, wxx_bi, Wxx_g), (Wd_d, wd_bi, Wd_g),
                               (Wh_d, wh_bi, Wh_g), (Wxy_d, wxy_bi, Wxy_g)):
                nc.gpsimd.dma_start(bi[:], src[:, :])
                nc.gpsimd.collective_compute(
                    "AllGather", ALU.bypass, replica_groups=RG,
                    ins=[bi.opt()], outs=[g.opt()])

            Wxx_sb = wpool.tile([128, KX * G4], mm_dt)
            Wxy_sb = wpool.tile([128, KY * G4], mm_dt)
            Wh_sb = wpool.tile([128, KH * G4], mm_dt)
            Wd_sb = wpool.tile([128, KH * O], mm_dt)
            # Matmult instructions can carry at most ONE sem wait in this
            # lowering; every matmul dependency must resolve to a single DVE
            # sem value. Weight DMAs are therefore "laundered" through
            # in-place DVE copies (one per DMA so each copy waits on one
            # DMA-queue sem only).
            for k in range(KX):
                nc.sync.dma_start(Wxx_sb[:, k * G4 : (k + 1) * G4],
                                  Wxx_g[k * 128 : (k + 1) * 128, :])
                nc.vector.tensor_copy(Wxx_sb[:, k * G4 : (k + 1) * G4],
                                      Wxx_sb[:, k * G4 : (k + 1) * G4])
            for k in range(KY):
                nc.sync.dma_start(Wxy_sb[:, k * G4 : (k + 1) * G4],
                                  Wxy_g[k * 128 : (k + 1) * 128, :])
                nc.vector.tensor_copy(Wxy_sb[:, k * G4 : (k + 1) * G4],
                                      Wxy_sb[:, k * G4 : (k + 1) * G4])
            for k in range(KH):
                nc.sync.dma_start(Wh_sb[:, k * G4 : (k + 1) * G4],
                                  Wh_g[k * 128 : (k + 1) * 128, :])
                nc.vector.tensor_copy(Wh_sb[:, k * G4 : (k + 1) * G4],
                                      Wh_sb[:, k * G4 : (k + 1) * G4])
                nc.sync.dma_start(Wd_sb[:, k * O : (k + 1) * O],
                                  Wd_g[k * 128 : (k + 1) * 128, :])
                nc.vector.tensor_copy(Wd_sb[:, k * O : (k + 1) * O],
                                      Wd_sb[:, k * O : (k + 1) * O])
            if use_bias_z:
                bz_sb = wpool.tile([128, 1024], F32)
                nc.sync.dma_start(bz_sb[:], bz_d[:, :])
            if use_bias_y:
                by_sb = wpool.tile([BL, O], F32)
                nc.sync.dma_start(by_sb[:], by_d[:, :])

            # identity for PE transposes (16-bit: f32 transpose-mode faults on
            # hw); I64 in both partition halves so the fmap can start at
            # partition 0 or 64 (must match the weights)
            ident = wpool.tile([128, 128], mm_dt)
            make_identity(nc, ident[:])
            nc.vector.tensor_copy(ident[:], ident[:])  # launder Pool dep -> DVE

            def emit_z_mms(z_tile, chunks, start, stop):
                nck = len(chunks)
                for ci, (lhsT, wtile, coff) in enumerate(chunks):
                    for beta in range(2):
                        for j in range(4):
                            w_lo = coff + 2048 * beta + 512 * j
                            nc.tensor.matmul(
                                z_tile[32 * j : 32 * (j + 1),
                                       512 * beta : 512 * (beta + 1)],
                                mc(lhsT),
                                mc(wtile[:, w_lo : w_lo + 512]),
                                start=(start and ci == 0),
                                stop=(stop and ci == nck - 1),
                                tile_position=(0, 32 * j),
                                skip_group_check=True,
                            )

            def load_x(t):
                xT_sb = xpool.tile([128, 2 * BL], mm_dt, name="xT_sb")
                nc.sync.dma_start(xT_sb[:], xT_d[t])
                # launder the x DMA-queue sem into the DVE sem
                xr_sb = xpool.tile([128, 2 * BL], mm_dt, name="xr_sb")
                nc.vector.tensor_copy(xr_sb[:], xT_sb[:])
                return [(xr_sb[:, bass.ts(k, BL)], Wxx_sb, k * G4)
                        for k in range(KX)]

            hT_prev = None
            yT_prev = None
            # software pipeline: the x-part of step t+1 is issued during step
            # t, so the in-order PE has independent work while the gate chain
            # (ACT/DVE) of step t runs.
            z_ps = zpsum.tile([128, 1024], F32, name="z_ps")
            emit_z_mms(z_ps, load_x(0), start=True, stop=(T_steps == 1))
            for t in range(T_steps):
                if t > 0:
                    # h first, y last: the y feedback chain (Wd+tanh+cast+
                    # transpose) of step t-1 gets the h-matmul span as slack
                    chunks = [(hT_prev[:, _hT_off(k) : _hT_off(k) + BL], Wh_sb,
                               k * G4) for k in (0, 2, 4, 6, 1, 3, 5, 7)]
                    chunks += [(yT_prev[:, bass.ts(k, BL)], Wxy_sb, k * G4)
                               for k in range(KY)]
                    emit_z_mms(z_ps, chunks, start=False, stop=True)
                if t + 1 < T_steps:
                    z_next = zpsum.tile([128, 1024], F32, name="z_ps")
                    emit_z_mms(z_next, load_x(t + 1), start=True, stop=False)
                else:
                    z_next = None

                # gate math: <=1 PSUM operand per DVE op
                if use_bias_z:
                    nc.vector.tensor_add(z_ps[:, 0:512], z_ps[:, 0:512],
                                         bz_sb[:, 0:512])
                    nc.vector.tensor_add(z_ps[:, 512:1024], z_ps[:, 512:1024],
                                         bz_sb[:, 512:1024])
                # gate chain split into column halves: half 0 finishes ->
                # its transpose + hT copy run while half 1 still computes, so
                # the even hT-chunk matmuls of step t+1 start earlier
                tg_sb = gpool.tile([128, 256], F32, name="tg_sb")
                o_sb = gpool.tile([128, 256], F32, name="o_sb")
                h_stk = gpool.tile([128, 256], mm_dt, name="h_stk")
                tr_ps = tpsum.tile([128, 320], mm_dt, name="tr_ps")
                hT_sb = hpool.tile([128, 256], mm_dt, name="hT_sb")
                for hf in range(2):
                    s = slice(128 * hf, 128 * hf + 128)
                    nc.scalar.activation(tg_sb[:, s], z_ps[:, 512 + 128 * hf :
                                                           640 + 128 * hf],
                                         AF.Tanh)
                    nc.scalar.activation(z_ps[:, s], z_ps[:, s], AF.Sigmoid)
                    nc.vector.tensor_mul(tg_sb[:, s], z_ps[:, s], tg_sb[:, s])
                    nc.scalar.activation(z_ps[:, 256 + 128 * hf : 384 + 128 * hf],
                                         z_ps[:, 256 + 128 * hf : 384 + 128 * hf],
                                         AF.Sigmoid)
                    nc.vector.tensor_mul(c_sb[:, s],
                                         z_ps[:, 256 + 128 * hf : 384 + 128 * hf],
                                         c_sb[:, s])
                    nc.scalar.activation(o_sb[:, s], z_ps[:, 768 + 128 * hf :
                                                          896 + 128 * hf],
                                         AF.Sigmoid)
                    nc.vector.tensor_add(c_sb[:, s], tg_sb[:, s], c_sb[:, s])
                    nc.scalar.activation(tg_sb[:, s], c_sb[:, s], AF.Tanh)
                    nc.vector.tensor_mul(h_stk[:, s], o_sb[:, s], tg_sb[:, s])
                    nc.tensor.transpose(tr_ps[:, s], h_stk[:, s], ident[:])
                    nc.vector.tensor_copy(hT_sb[:, s], tr_ps[:, s])

                # y = tanh(h @ Wd + bd)
                y_ps = ypsum.tile([BL, O], F32, name="y_ps")
                for k in range(KH):
                    nc.tensor.matmul(
                        y_ps[:],
                        mc(hT_sb[:, _hT_off(k) : _hT_off(k) + BL]),
                        mc(Wd_sb[:, k * O : (k + 1) * O]),
                        start=(k == 0),
                        stop=(k == KH - 1),
                    )
                if use_bias_y:
                    nc.vector.tensor_add(y_ps[:], y_ps[:], by_sb[:])
                y_sb = ypool.tile([BL, O], F32, name="y_sb", bufs=2)
                nc.scalar.activation(y_sb[:], y_ps[:], AF.Tanh)
                # cast y for the fp16 PE-transposes (also launders ACT -> DVE);
                # this is on the feedback critical path, so it runs before the
                # u8 output quantization
                y_bf = ypool.tile([BL, O], mm_dt, name="y_bf")
                nc.vector.tensor_copy(y_bf[:], y_sb[:])
                # own double-buffered tile so the output DMA never blocks
                # the next step's gate ACTs
                if out_u8:
                    y_out = ypool.tile([BL, O], U8, name="y_out", bufs=2)
                    nc.vector.tensor_scalar(y_out[:], y_sb[:], 127.0, u8_bias,
                                            ALU.mult, ALU.add)
                else:
                    y_out = ypool.tile([BL, O], mm_dt, name="y_out", bufs=2)
                    nc.vector.tensor_copy(y_out[:], y_sb[:])
                nc.sync.dma_start(ys_d[:, t, :], y_out[:])

                # y -> yT via 2 PE transposes
                for q in range(2):
                    nc.tensor.transpose(
                        tr_ps[:, 256 + 32 * q : 256 + 32 * (q + 1)],
                        y_bf[0:BL, 128 * q : 128 * (q + 1)],
                        ident[0:32, 0:32],
                    )
                yT_sb = ypool.tile([128, 2 * BL], mm_dt, name="yT_sb")
                nc.vector.tensor_copy(yT_sb[:], tr_ps[:, 256:320])

                hT_prev = hT_sb
                yT_prev = yT_sb
                z_ps = z_next

    nc.compile()
    return nc


def prep_inputs(x, Wx, Wh, b, Wd, bd, T_steps: int = T,
                mm_np=np.float16):
    """Host-side shard + relayout. Returns (in_maps, use_bias_z, use_bias_y)."""
    x = np.asarray(x, dtype=np.float32)[:, :T_steps, :]
    Wx = np.asarray(Wx, dtype=np.float32)
    Wh = np.asarray(Wh, dtype=np.float32)
    b = np.asarray(b, dtype=np.float32)
    Wd = np.asarray(Wd, dtype=np.float32)
    bd = np.asarray(bd, dtype=np.float32)

    perm = gate_perm()
    Wxp = np.ascontiguousarray(Wx[:, perm]).astype(mm_np)
    Whp = np.ascontiguousarray(Wh[:, perm]).astype(mm_np)
    Wxx = np.ascontiguousarray(Wxp[:D])
    Wxy = np.ascontiguousarray(Wxp[D:])
    Wd = Wd.astype(mm_np)

    use_bias_z = bool(np.any(b))
    use_bias_y = bool(np.any(bd))
    shared = {}
    if use_bias_z:
        bp = b[perm]
        bz = np.empty((128, 1024), dtype=np.float32)
        for j in range(4):
            for beta in range(2):
                bz[32 * j : 32 * (j + 1), 512 * beta : 512 * (beta + 1)] = bp[
                    2048 * beta + 512 * j : 2048 * beta + 512 * j + 512][None, :]
        shared["bz"] = bz
    if use_bias_y:
        shared["by"] = np.broadcast_to(bd, (BL, O)).copy()

    in_maps = []
    for c in range(NCORES):
        xc = x[c * BL : (c + 1) * BL]                      # [BL, T, D]
        xT = xc.transpose(1, 2, 0)                         # [T, D, BL]
        xT = xT.reshape(T_steps, 2, 128, BL).transpose(0, 2, 1, 3)
        xT = np.ascontiguousarray(
            xT.reshape(T_steps, 128, 2 * BL)).astype(mm_np)
        in_maps.append({
            "xT": xT,
            "Wxx": np.ascontiguousarray(Wxx[c * WXS : (c + 1) * WXS]),
            "Wxy": np.ascontiguousarray(Wxy[c * WXS : (c + 1) * WXS]),
            "Wh": np.ascontiguousarray(Whp[c * WHS : (c + 1) * WHS]),
            "Wd": np.ascontiguousarray(Wd[c * WHS : (c + 1) * WHS]),
            **shared,
        })
    return in_maps, use_bias_z, use_bias_y


def decode_ys(res, out_u8: bool = True):
    """Concatenate per-core results and decode to fp32 [B, T, O]."""
    parts = []
    for c in range(NCORES):
        ys = res.results[c]["ys"]
        if out_u8:
            parts.append((ys.astype(np.float32) - 128.0) * (1.0 / 127.0))
        else:
            parts.append(ys.astype(np.float32))
    return np.concatenate(parts, axis=0)


def kernel(x, Wx, Wh, b, Wd, bd):
    from concourse.bass_utils import run_bass_kernel_spmd

    in_maps, ubz, uby = prep_inputs(x, Wx, Wh, b, Wd, bd, T)
    nc = build_nc(T, ubz, uby)
    res = run_bass_kernel_spmd(nc, in_maps, list(range(NCORES)))
    return decode_ys(res)


# revision 14
# speedup vs baseline: 4.8035x; 3.1874x over previous
"""Autoregressive LSTM cell (B=256, T=256, D=256, H=1024, O=256) on 8 TRN2 cores.

Strategy: pure data-parallel over batch (32 rows/core). The end-to-end time is
dominated by host<->device transfer over the axon tunnel (~58MB/s h2d,
~42MB/s d2h), not device compute (~1.5ms), so the kernel minimizes shipped
bytes:
  - x is shipped fp16 (4MB/core);
  - weights are shipped SHARDED 1/8 per core (fp16, ~1.6MB/core) and
    reassembled on device with four DRAM AllGather collectives;
  - the output is shipped uint8: u = round(127*y) + 128, decoded on host as
    (u-128)/127 (2MB/core out + 2MB/core donated zero-buffer in).
The whole matmul/feedback pipeline runs fp16 (vs bf16 before), which drops
the fp32-reference error to ~9e-4 and leaves the u8 output quantization
(~4e-3) well inside the 2e-2 gate.

Per step t (sequential, 256 steps):
    z = x_t @ Wxx + y_{t-1} @ Wxy + h_{t-1} @ Wh     (+b)
    i,f,g,o gates -> c = sig(f)*c + sig(i)*tanh(g); h = sig(o)*tanh(c)
    y = tanh(h @ Wd + bd)
Matmuls are "activation-stationary": lhsT = activation^T [K<=128, M=32batch],
rhs streams fp16 weight columns at 1 column/cycle (fp32 PSUM accumulation;
fp32 gate math). 4-way PE column tiling (tile_position col groups) packs 4
batch-32 matmuls concurrently, writing z in a stacked PSUM layout:
  z_ps [128, 1024]: position (32j+b, 512*beta + n) = z_perm[2048*beta + 512j + n]
Weight columns are host-permuted so that
  bank0 (cols 0:512)  = [ sig-gate i | sig-gate f ] column-paired per channel
  bank1 (cols 512:1024) = [ tanh-gate g | sig-gate o ]
with channel(p=32j+b, n) = 256j + (n mod 256): all gate elementwise ops are
partition-aligned and the c update is a free-dim-shifted add.
h/y are fed back transposed via PE transpose-mode matmuls.

Overlap structure: the x-part matmuls of step t+1 are software-pipelined into
step t, z-chunks are ordered x->h->y so the y-feedback chain hides under the
h-matmul span, and the gate chain runs in two column halves.
"""

import sys

for p in ("/opt/trn_rl_repo",):
    if p not in sys.path:
        sys.path.insert(0, p)

from contextlib import ExitStack

import numpy as np

import concourse.bacc as bacc
import concourse.bass as bass
import concourse.mybir as mybir
import concourse.tile as tile
from concourse.bass import ds
from concourse.masks import make_identity

F32 = mybir.dt.float32
U8 = mybir.dt.uint8
AF = mybir.ActivationFunctionType
ALU = mybir.AluOpType

B, T, D, H, O = 256, 256, 256, 1024, 256
NCORES = 8
BL = B // NCORES  # 32
G4 = 4 * H  # 4096
KX, KY, KH = D // 128, O // 128, H // 128  # 2, 2, 8
WXS = D // NCORES  # 32 weight-shard rows per core for Wxx/Wxy
WHS = H // NCORES  # 128 shard rows for Wh/Wd

# u8 output encoding: u = convert_u8(127*y + U8_BIAS), decoded (u-128)/127.
# The DVE float->u8 convert rounds-to-nearest (measured on hw: mean code
# offset -0.001, std 0.289), so the bias is exactly 128.0.
U8_BIAS_DEFAULT = 128.0


def gate_perm() -> np.ndarray:
    """Map stored z column position -> original gate column (i,f,g,o order)."""
    perm = np.empty(G4, dtype=np.int64)
    for beta in (0, 1):
        for j in range(4):
            for half in (0, 1):
                gate = (0, 1, 2, 3)[2 * beta + half]
                src = 1024 * gate + 256 * j
                pos = 2048 * beta + 512 * j + 256 * half
                perm[pos : pos + 256] = np.arange(src, src + 256)
    return perm


def _hT_off(c: int) -> int:
    """Column offset of h^T chunk c (channels 128c:128c+128) inside hT_sb."""
    return 128 * (c % 2) + 32 * (c // 2)


def build_nc(T_steps: int = T, use_bias_z: bool = False, use_bias_y: bool = False,
             mm_dt=mybir.dt.float16, out_u8: bool = True,
             u8_bias: float = U8_BIAS_DEFAULT):
    nc = bacc.Bacc()

    xT_d = nc.declare_dram_parameter("xT", [T_steps, 128, 2 * BL], mm_dt,
                                     isOutput=False)
    # weights arrive as row-shards: core c holds rows [c*WXS:(c+1)*WXS] of
    # Wxx/Wxy and rows [c*WHS:(c+1)*WHS] of Wh/Wd; AllGather rebuilds the
    # full matrices in DRAM (saves 7/8 of the weight bytes over the tunnel).
    Wxx_d = nc.declare_dram_parameter("Wxx", [WXS, G4], mm_dt, isOutput=False)
    Wxy_d = nc.declare_dram_parameter("Wxy", [WXS, G4], mm_dt, isOutput=False)
    Wh_d = nc.declare_dram_parameter("Wh", [WHS, G4], mm_dt, isOutput=False)
    Wd_d = nc.declare_dram_parameter("Wd", [WHS, O], mm_dt, isOutput=False)
    bz_d = by_d = None
    if use_bias_z:
        bz_d = nc.declare_dram_parameter("bz", [128, 1024], F32, isOutput=False)
    if use_bias_y:
        by_d = nc.declare_dram_parameter("by", [BL, O], F32, isOutput=False)
    out_dt = U8 if out_u8 else mm_dt
    # t-major so the per-step store is one outer-dim (dynamic) slice
    ys_d = nc.declare_dram_parameter("ys", [T_steps, BL, O], out_dt,
                                     isOutput=True)

    def mc(ap):
        return ap.bitcast(mm_dt) if ap.dtype != mm_dt else ap

    with tile.TileContext(nc) as tc:
        with ExitStack() as ctx:
            dpool = ctx.enter_context(
                tc.tile_pool(name="dramw", bufs=1, space="DRAM"))
            wpool = ctx.enter_context(tc.tile_pool(name="weights", bufs=1))
            state = ctx.enter_context(tc.tile_pool(name="state", bufs=1))
            xpool = ctx.enter_context(tc.tile_pool(name="xin", bufs=3))
            gpool = ctx.enter_context(tc.tile_pool(name="gates", bufs=1))
            hpool = ctx.enter_context(tc.tile_pool(name="hT", bufs=1))
            ypool = ctx.enter_context(tc.tile_pool(name="yt", bufs=1))
            zpsum = ctx.enter_context(tc.tile_pool(name="zps", bufs=2, space="PSUM"))
            ypsum = ctx.enter_context(tc.tile_pool(name="yps", bufs=2, space="PSUM"))
            tpsum = ctx.enter_context(tc.tile_pool(name="tps", bufs=2, space="PSUM"))

            # c state, channel(32j+b, n) = 256j + n  (memset first: needed at
            # t=0, and it shares the gpsimd queue with the gathers below)
            c_sb = state.tile([128, 256], F32)
            nc.gpsimd.memset(c_sb[:], 0.0)

            # ---- on-device weight reassembly: DRAM AllGather per matrix ----
            # bounce buffers: collectives can't operate on I/O tensors.
            wxx_bi = dpool.tile([WXS, G4], mm_dt, name="wxx_bi")
            wxy_bi = dpool.tile([WXS, G4], mm_dt, name="wxy_bi")
            wh_bi = dpool.tile([WHS, G4], mm_dt, name="wh_bi")
            wd_bi = dpool.tile([WHS, O], mm_dt, name="wd_bi")
            Wxx_g = dpool.tile([D, G4], mm_dt, name="Wxx_g")
            Wxy_g = dpool.tile([O, G4], mm_dt, name="Wxy_g")
            Wh_g = dpool.tile([H, G4], mm_dt, name="Wh_g")
            Wd_g = dpool.tile([H, O], mm_dt, name="Wd_g")
            RG = [list(range(NCORES))]
            # gather order = first-use order: Wxx (t=0 z), Wd (t=0 y), Wh/Wxy
            # (t=1 z)
            for src, bi, g in ((Wxx_d, wxx_bi, Wxx_g), (Wd_d, wd_bi, Wd_g),
                               (Wh_d, wh_bi, Wh_g), (Wxy_d, wxy_bi, Wxy_g)):
                nc.gpsimd.dma_start(bi[:], src[:, :])
                nc.gpsimd.collective_compute(
                    "AllGather", ALU.bypass, replica_groups=RG,
                    ins=[bi.opt()], outs=[g.opt()])

            Wxx_sb = wpool.tile([128, KX * G4], mm_dt)
            Wxy_sb = wpool.tile([128, KY * G4], mm_dt)
            Wh_sb = wpool.tile([128, KH * G4], mm_dt)
            Wd_sb = wpool.tile([128, KH * O], mm_dt)
            # Matmult instructions can carry at most ONE sem wait in this
            # lowering; every matmul dependency must resolve to a single DVE
            # sem value. Weight DMAs are therefore "laundered" through
            # in-place DVE copies (one per DMA so each copy waits on one
            # DMA-queue sem only).
            for k in range(KX):
                nc.sync.dma_start(Wxx_sb[:, k * G4 : (k + 1) * G4],
                                  Wxx_g[k * 128 : (k + 1) * 128, :])
                nc.vector.tensor_copy(Wxx_sb[:, k * G4 : (k + 1) * G4],
                                      Wxx_sb[:, k * G4 : (k + 1) * G4])
            for k in range(KY):
                nc.sync.dma_start(Wxy_sb[:, k * G4 : (k + 1) * G4],
                                  Wxy_g[k * 128 : (k + 1) * 128, :])
                nc.vector.tensor_copy(Wxy_sb[:, k * G4 : (k + 1) * G4],
                                      Wxy_sb[:, k * G4 : (k + 1) * G4])
            for k in range(KH):
                nc.sync.dma_start(Wh_sb[:, k * G4 : (k + 1) * G4],
                                  Wh_g[k * 128 : (k + 1) * 128, :])
                nc.vector.tensor_copy(Wh_sb[:, k * G4 : (k + 1) * G4],
                                      Wh_sb[:, k * G4 : (k + 1) * G4])
                nc.sync.dma_start(Wd_sb[:, k * O : (k + 1) * O],
                                  Wd_g[k * 128 : (k + 1) * 128, :])
                nc.vector.tensor_copy(Wd_sb[:, k * O : (k + 1) * O],
                                      Wd_sb[:, k * O : (k + 1) * O])
            if use_bias_z:
                bz_sb = wpool.tile([128, 1024], F32)
                nc.sync.dma_start(bz_sb[:], bz_d[:, :])
            if use_bias_y:
                by_sb = wpool.tile([BL, O], F32)
                nc.sync.dma_start(by_sb[:], by_d[:, :])

            # identity for PE transposes (16-bit: f32 transpose-mode faults on
            # hw); I64 in both partition halves so the fmap can start at
            # partition 0 or 64 (must match the weights)
            ident = wpool.tile([128, 128], mm_dt)
            make_identity(nc, ident[:])
            nc.vector.tensor_copy(ident[:], ident[:])  # launder Pool dep -> DVE

            def emit_z_mms(z_tile, chunks, start, stop):
                nck = len(chunks)
                for ci, (lhsT, wtile, coff) in enumerate(chunks):
                    for beta in range(2):
                        for j in range(4):
                            w_lo = coff + 2048 * beta + 512 * j
                            nc.tensor.matmul(
                                z_tile[32 * j : 32 * (j + 1),
                                       512 * beta : 512 * (beta + 1)],
                                mc(lhsT),
                                mc(wtile[:, w_lo : w_lo + 512]),
                                start=(start and ci == 0),
                                stop=(stop and ci == nck - 1),
                                tile_position=(0, 32 * j),
                                skip_group_check=True,
                            )

            def load_x(idx):
                """idx: python int or ScalarValue (dynamic) step index."""
                xT_sb = xpool.tile([128, 2 * BL], mm_dt, name="xT_sb")
                if isinstance(idx, int):
                    nc.sync.dma_start(xT_sb[:], xT_d[idx])
                else:
                    nc.sync.dma_start(xT_sb[:], xT_d[ds(idx, 1)].squeeze(0))
                # launder the x DMA-queue sem into the DVE sem
                xr_sb = xpool.tile([128, 2 * BL], mm_dt, name="xr_sb")
                nc.vector.tensor_copy(xr_sb[:], xT_sb[:])
                return [(xr_sb[:, bass.ts(k, BL)], Wxx_sb, k * G4)
                        for k in range(KX)]

            # Loop-carried feedback state must be FIXED tiles written in
            # place (like c_sb): per-iteration pool allocations read via a
            # pre-loop handle deadlock the tile scheduler at the back edge.
            # The uniform loop body always runs the h/y matmuls, so step 0
            # consumes the memset h_{-1}=y_{-1}=0 state.
            hT_state = hpool.tile([128, 256], mm_dt, name="hT_st")
            nc.gpsimd.memset(hT_state[:], 0.0)
            yT_state = ypool.tile([128, 2 * BL], mm_dt, name="yT_st")
            nc.gpsimd.memset(yT_state[:], 0.0)

            UNROLL = 8
            assert T_steps % UNROLL == 0

            # software pipeline: within a group, the x-part of step t+1 is
            # issued during step t, so the in-order PE has independent work
            # while the gate chain (ACT/DVE) of step t runs. The pipeline
            # restarts at each group boundary (the loop back-edge is a full
            # barrier), costing a few us per group.
            with tc.For_i(0, T_steps, UNROLL) as t0:
                z_ps = zpsum.tile([128, 1024], F32, name="z_ps")
                emit_z_mms(z_ps, load_x(t0), start=True, stop=False)
                for j in range(UNROLL):
                    # h first, y last: the y feedback chain (Wd+tanh+cast+
                    # transpose) of step t-1 gets the h-matmul span as slack
                    chunks = [(hT_state[:, _hT_off(k) : _hT_off(k) + BL],
                               Wh_sb, k * G4) for k in (0, 2, 4, 6, 1, 3, 5, 7)]
                    chunks += [(yT_state[:, bass.ts(k, BL)], Wxy_sb, k * G4)
                               for k in range(KY)]
                    emit_z_mms(z_ps, chunks, start=False, stop=True)
                    if j + 1 < UNROLL:
                        z_next = zpsum.tile([128, 1024], F32, name="z_ps")
                        emit_z_mms(z_next, load_x(t0 + (j + 1)), start=True,
                                   stop=False)
                    else:
                        z_next = None

                    # gate math: <=1 PSUM operand per DVE op
                    if use_bias_z:
                        nc.vector.tensor_add(z_ps[:, 0:512], z_ps[:, 0:512],
                                             bz_sb[:, 0:512])
                        nc.vector.tensor_add(z_ps[:, 512:1024],
                                             z_ps[:, 512:1024],
                                             bz_sb[:, 512:1024])
                    # gate chain split into column halves: half 0 finishes ->
                    # its transpose + hT copy run while half 1 still computes,
                    # so the even hT-chunk matmuls of step t+1 start earlier
                    tg_sb = gpool.tile([128, 256], F32, name="tg_sb")
                    o_sb = gpool.tile([128, 256], F32, name="o_sb")
                    h_stk = gpool.tile([128, 256], mm_dt, name="h_stk")
                    tr_ps = tpsum.tile([128, 320], mm_dt, name="tr_ps")
                    hT_sb = hT_state
                    for hf in range(2):
                        s = slice(128 * hf, 128 * hf + 128)
                        nc.scalar.activation(tg_sb[:, s],
                                             z_ps[:, 512 + 128 * hf :
                                                  640 + 128 * hf],
                                             AF.Tanh)
                        nc.scalar.activation(z_ps[:, s], z_ps[:, s], AF.Sigmoid)
                        nc.vector.tensor_mul(tg_sb[:, s], z_ps[:, s],
                                             tg_sb[:, s])
                        nc.scalar.activation(z_ps[:, 256 + 128 * hf :
                                                  384 + 128 * hf],
                                             z_ps[:, 256 + 128 * hf :
                                                  384 + 128 * hf],
                                             AF.Sigmoid)
                        nc.vector.tensor_mul(c_sb[:, s],
                                             z_ps[:, 256 + 128 * hf :
                                                  384 + 128 * hf],
                                             c_sb[:, s])
                        nc.scalar.activation(o_sb[:, s],
                                             z_ps[:, 768 + 128 * hf :
                                                  896 + 128 * hf],
                                             AF.Sigmoid)
                        nc.vector.tensor_add(c_sb[:, s], tg_sb[:, s],
                                             c_sb[:, s])
                        nc.scalar.activation(tg_sb[:, s], c_sb[:, s], AF.Tanh)
                        nc.vector.tensor_mul(h_stk[:, s], o_sb[:, s],
                                             tg_sb[:, s])
                        nc.tensor.transpose(tr_ps[:, s], h_stk[:, s], ident[:])
                        nc.vector.tensor_copy(hT_sb[:, s], tr_ps[:, s])

                    # y = tanh(h @ Wd + bd)
                    y_ps = ypsum.tile([BL, O], F32, name="y_ps")
                    for k in range(KH):
                        nc.tensor.matmul(
                            y_ps[:],
                            mc(hT_sb[:, _hT_off(k) : _hT_off(k) + BL]),
                            mc(Wd_sb[:, k * O : (k + 1) * O]),
                            start=(k == 0),
                            stop=(k == KH - 1),
                        )
                    if use_bias_y:
                        nc.vector.tensor_add(y_ps[:], y_ps[:], by_sb[:])
                    y_sb = ypool.tile([BL, O], F32, name="y_sb", bufs=2)
                    nc.scalar.activation(y_sb[:], y_ps[:], AF.Tanh)
                    # cast y for the fp16 PE-transposes (also launders
                    # ACT -> DVE); this is on the feedback critical path, so
                    # it runs before the u8 output quantization
                    y_bf = ypool.tile([BL, O], mm_dt, name="y_bf")
                    nc.vector.tensor_copy(y_bf[:], y_sb[:])
                    # own double-buffered tile so the output DMA never blocks
                    # the next step's gate ACTs
                    if out_u8:
                        y_out = ypool.tile([BL, O], U8, name="y_out", bufs=2)
                        nc.vector.tensor_scalar(y_out[:], y_sb[:], 127.0,
                                                u8_bias, ALU.mult, ALU.add)
                    else:
                        y_out = ypool.tile([BL, O], mm_dt, name="y_out",
                                           bufs=2)
                        nc.vector.tensor_copy(y_out[:], y_sb[:])
                    nc.sync.dma_start(ys_d[ds(t0 + j, 1)].squeeze(0), y_out[:])

                    # y -> yT via 2 PE transposes
                    for q in range(2):
                        nc.tensor.transpose(
                            tr_ps[:, 256 + 32 * q : 256 + 32 * (q + 1)],
                            y_bf[0:BL, 128 * q : 128 * (q + 1)],
                            ident[0:32, 0:32],
                        )
                    nc.vector.tensor_copy(yT_state[:], tr_ps[:, 256:320])

                    z_ps = z_next

    nc.compile()
    return nc


def prep_inputs(x, Wx, Wh, b, Wd, bd, T_steps: int = T,
                mm_np=np.float16):
    """Host-side shard + relayout. Returns (in_maps, use_bias_z, use_bias_y)."""
    x = np.asarray(x, dtype=np.float32)[:, :T_steps, :]
    Wx = np.asarray(Wx, dtype=np.float32)
    Wh = np.asarray(Wh, dtype=np.float32)
    b = np.asarray(b, dtype=np.float32)
    Wd = np.asarray(Wd, dtype=np.float32)
    bd = np.asarray(bd, dtype=np.float32)

    perm = gate_perm()
    Wxp = np.ascontiguousarray(Wx[:, perm]).astype(mm_np)
    Whp = np.ascontiguousarray(Wh[:, perm]).astype(mm_np)
    Wxx = np.ascontiguousarray(Wxp[:D])
    Wxy = np.ascontiguousarray(Wxp[D:])
    Wd = Wd.astype(mm_np)

    use_bias_z = bool(np.any(b))
    use_bias_y = bool(np.any(bd))
    shared = {}
    if use_bias_z:
        bp = b[perm]
        bz = np.empty((128, 1024), dtype=np.float32)
        for j in range(4):
            for beta in range(2):
                bz[32 * j : 32 * (j + 1), 512 * beta : 512 * (beta + 1)] = bp[
                    2048 * beta + 512 * j : 2048 * beta + 512 * j + 512][None, :]
        shared["bz"] = bz
    if use_bias_y:
        shared["by"] = np.broadcast_to(bd, (BL, O)).copy()

    in_maps = []
    for c in range(NCORES):
        xc = x[c * BL : (c + 1) * BL]                      # [BL, T, D]
        xT = xc.transpose(1, 2, 0)                         # [T, D, BL]
        xT = xT.reshape(T_steps, 2, 128, BL).transpose(0, 2, 1, 3)
        xT = np.ascontiguousarray(
            xT.reshape(T_steps, 128, 2 * BL)).astype(mm_np)
        in_maps.append({
            "xT": xT,
            "Wxx": np.ascontiguousarray(Wxx[c * WXS : (c + 1) * WXS]),
            "Wxy": np.ascontiguousarray(Wxy[c * WXS : (c + 1) * WXS]),
            "Wh": np.ascontiguousarray(Whp[c * WHS : (c + 1) * WHS]),
            "Wd": np.ascontiguousarray(Wd[c * WHS : (c + 1) * WHS]),
            **shared,
        })
    return in_maps, use_bias_z, use_bias_y


def decode_ys(res, out_u8: bool = True):
    """Concatenate per-core results and decode to fp32 [B, T, O]."""
    parts = []
    for c in range(NCORES):
        ys = res.results[c]["ys"]                 # [T, BL, O] t-major
        ys = np.ascontiguousarray(ys.transpose(1, 0, 2))  # [BL, T, O]
        if out_u8:
            parts.append((ys.astype(np.float32) - 128.0) * (1.0 / 127.0))
        else:
            parts.append(ys.astype(np.float32))
    return np.concatenate(parts, axis=0)


def kernel(x, Wx, Wh, b, Wd, bd):
    from concourse.bass_utils import run_bass_kernel_spmd

    in_maps, ubz, uby = prep_inputs(x, Wx, Wh, b, Wd, bd, T)
    nc = build_nc(T, ubz, uby)
    res = run_bass_kernel_spmd(nc, in_maps, list(range(NCORES)))
    return decode_ys(res)


# revision 22
# speedup vs baseline: 5.1636x; 1.0749x over previous
"""Autoregressive LSTM cell (B=256, T=256, D=256, H=1024, O=256) on 8 TRN2 cores.

Strategy: pure data-parallel over batch (32 rows/core). The end-to-end time is
dominated by host<->device transfer over the axon tunnel (~58MB/s h2d,
~42MB/s d2h), not device compute (~1.5ms), so the kernel minimizes shipped
bytes:
  - x is shipped fp16 (4MB/core);
  - weights are shipped SHARDED 1/8 per core (fp16, ~1.6MB/core) and
    reassembled on device with four DRAM AllGather collectives;
  - the output is shipped uint8: u = round(127*y) + 128, decoded on host as
    (u-128)/127 (2MB/core out + 2MB/core donated zero-buffer in).
The whole matmul/feedback pipeline runs fp16 (vs bf16 before), which drops
the fp32-reference error to ~9e-4 and leaves the u8 output quantization
(~4e-3) well inside the 2e-2 gate.

Per step t (sequential, 256 steps):
    z = x_t @ Wxx + y_{t-1} @ Wxy + h_{t-1} @ Wh     (+b)
    i,f,g,o gates -> c = sig(f)*c + sig(i)*tanh(g); h = sig(o)*tanh(c)
    y = tanh(h @ Wd + bd)
Matmuls are "activation-stationary": lhsT = activation^T [K<=128, M=32batch],
rhs streams fp16 weight columns at 1 column/cycle (fp32 PSUM accumulation;
fp32 gate math). 4-way PE column tiling (tile_position col groups) packs 4
batch-32 matmuls concurrently, writing z in a stacked PSUM layout:
  z_ps [128, 1024]: position (32j+b, 512*beta + n) = z_perm[2048*beta + 512j + n]
Weight columns are host-permuted so that
  bank0 (cols 0:512)  = [ sig-gate i | sig-gate f ] column-paired per channel
  bank1 (cols 512:1024) = [ tanh-gate g | sig-gate o ]
with channel(p=32j+b, n) = 256j + (n mod 256): all gate elementwise ops are
partition-aligned and the c update is a free-dim-shifted add.
h/y are fed back transposed via PE transpose-mode matmuls.

Overlap structure: the x-part matmuls of step t+1 are software-pipelined into
step t, z-chunks are ordered x->h->y so the y-feedback chain hides under the
h-matmul span, and the gate chain runs in two column halves.
"""

import sys

for p in ("/opt/trn_rl_repo",):
    if p not in sys.path:
        sys.path.insert(0, p)

from contextlib import ExitStack

import numpy as np

import concourse.bacc as bacc
import concourse.bass as bass
import concourse.mybir as mybir
import concourse.tile as tile
from concourse.bass import ds
from concourse.masks import make_identity

F32 = mybir.dt.float32
U8 = mybir.dt.uint8
AF = mybir.ActivationFunctionType
ALU = mybir.AluOpType

B, T, D, H, O = 256, 256, 256, 1024, 256
NCORES = 8
BL = B // NCORES  # 32
G4 = 4 * H  # 4096
KX, KY, KH = D // 128, O // 128, H // 128  # 2, 2, 8
WXS = D // NCORES  # 32 weight-shard rows per core for Wxx/Wxy
WHS = H // NCORES  # 128 shard rows for Wh/Wd
# flat per-core weight shard: [Wxx | Wxy | Wh | Wd] row-shards, one tensor so
# the tunnel pays one per-array transfer latency instead of four
WOFF = (0, WXS * G4, 2 * WXS * G4, 2 * WXS * G4 + WHS * G4,
        2 * WXS * G4 + WHS * G4 + WHS * O)
WSH_N = WOFF[4]  # 819200
# x ships as 12-bit fixed point: u = round(x/s) + 2048, s = max|x|/2047,
# packed as byte planes [B0 | B1 | B2] (2 values -> 3 bytes). The device
# unpacks to the integer v = u - 2048 in fp16; s is folded into Wxx host-side.
X12 = True

# u8 output encoding: u = convert_u8(127*y + U8_BIAS), decoded (u-128)/127.
# The DVE float->u8 convert rounds-to-nearest (measured on hw: mean code
# offset -0.001, std 0.289), so the bias is exactly 128.0.
U8_BIAS_DEFAULT = 128.0


def gate_perm() -> np.ndarray:
    """Map stored z column position -> original gate column (i,f,g,o order)."""
    perm = np.empty(G4, dtype=np.int64)
    for beta in (0, 1):
        for j in range(4):
            for half in (0, 1):
                gate = (0, 1, 2, 3)[2 * beta + half]
                src = 1024 * gate + 256 * j
                pos = 2048 * beta + 512 * j + 256 * half
                perm[pos : pos + 256] = np.arange(src, src + 256)
    return perm


def _hT_off(c: int) -> int:
    """Column offset of h^T chunk c (channels 128c:128c+128) inside hT_sb."""
    return 128 * (c % 2) + 32 * (c // 2)


def build_nc(T_steps: int = T, use_bias_z: bool = False, use_bias_y: bool = False,
             mm_dt=mybir.dt.float16, out_u8: bool = True,
             u8_bias: float = U8_BIAS_DEFAULT):
    nc = bacc.Bacc()

    if X12:
        xT_d = nc.declare_dram_parameter("xT", [T_steps, 128, 3 * BL], U8,
                                         isOutput=False)
    else:
        xT_d = nc.declare_dram_parameter("xT", [T_steps, 128, 2 * BL], mm_dt,
                                         isOutput=False)
    # weights arrive as one flat row-shard: core c holds rows
    # [c*WXS:(c+1)*WXS] of Wxx/Wxy and rows [c*WHS:(c+1)*WHS] of Wh/Wd;
    # AllGather rebuilds the full matrices in DRAM (saves 7/8 of the weight
    # bytes over the tunnel).
    wsh_d = nc.declare_dram_parameter("wsh", [WSH_N], mm_dt, isOutput=False)
    bz_d = by_d = None
    if use_bias_z:
        bz_d = nc.declare_dram_parameter("bz", [128, 1024], F32, isOutput=False)
    if use_bias_y:
        by_d = nc.declare_dram_parameter("by", [BL, O], F32, isOutput=False)
    out_dt = U8 if out_u8 else mm_dt
    # t-major so the per-step store is one outer-dim (dynamic) slice
    ys_d = nc.declare_dram_parameter("ys", [T_steps, BL, O], out_dt,
                                     isOutput=True)

    def mc(ap):
        return ap.bitcast(mm_dt) if ap.dtype != mm_dt else ap

    with tile.TileContext(nc) as tc:
        with ExitStack() as ctx:
            dpool = ctx.enter_context(
                tc.tile_pool(name="dramw", bufs=1, space="DRAM"))
            wpool = ctx.enter_context(tc.tile_pool(name="weights", bufs=1))
            state = ctx.enter_context(tc.tile_pool(name="state", bufs=1))
            xpool = ctx.enter_context(tc.tile_pool(name="xin", bufs=3))
            gpool = ctx.enter_context(tc.tile_pool(name="gates", bufs=1))
            hpool = ctx.enter_context(tc.tile_pool(name="hT", bufs=1))
            ypool = ctx.enter_context(tc.tile_pool(name="yt", bufs=1))
            zpsum = ctx.enter_context(tc.tile_pool(name="zps", bufs=2, space="PSUM"))
            ypsum = ctx.enter_context(tc.tile_pool(name="yps", bufs=2, space="PSUM"))
            tpsum = ctx.enter_context(tc.tile_pool(name="tps", bufs=2, space="PSUM"))

            # c state, channel(32j+b, n) = 256j + n  (memset first: needed at
            # t=0, and it shares the gpsimd queue with the gathers below)
            c_sb = state.tile([128, 256], F32)
            nc.gpsimd.memset(c_sb[:], 0.0)

            # ---- on-device weight reassembly: DRAM AllGather per matrix ----
            # bounce buffer: collectives can't operate on I/O tensors. One
            # bounce DMA, then four gathers reading flat slices of it; each
            # gather's output lands in the matrix's natural row-major layout
            # (rank r's rows land at block r).
            wsh_b = dpool.tile([WSH_N], mm_dt, name="wsh_b")
            nc.gpsimd.dma_start(wsh_b[:], wsh_d[:])
            Wxx_g = dpool.tile([D, G4], mm_dt, name="Wxx_g")
            Wxy_g = dpool.tile([O, G4], mm_dt, name="Wxy_g")
            Wh_g = dpool.tile([H, G4], mm_dt, name="Wh_g")
            Wd_g = dpool.tile([H, O], mm_dt, name="Wd_g")
            RG = [list(range(NCORES))]
            # gather order = first-use order: Wxx (t=0 z), Wd (t=0 y), Wh/Wxy
            # (t=1 z)
            for (a, b), g in (((WOFF[0], WOFF[1]), Wxx_g),
                              ((WOFF[3], WOFF[4]), Wd_g),
                              ((WOFF[2], WOFF[3]), Wh_g),
                              ((WOFF[1], WOFF[2]), Wxy_g)):
                nc.gpsimd.collective_compute(
                    "AllGather", ALU.bypass, replica_groups=RG,
                    ins=[wsh_b[a:b].opt()], outs=[g.opt()])

            Wxx_sb = wpool.tile([128, KX * G4], mm_dt)
            Wxy_sb = wpool.tile([128, KY * G4], mm_dt)
            Wh_sb = wpool.tile([128, KH * G4], mm_dt)
            Wd_sb = wpool.tile([128, KH * O], mm_dt)
            # Matmult instructions can carry at most ONE sem wait in this
            # lowering; every matmul dependency must resolve to a single DVE
            # sem value. Weight DMAs are therefore "laundered" through
            # in-place DVE copies (one per DMA so each copy waits on one
            # DMA-queue sem only).
            for k in range(KX):
                nc.sync.dma_start(Wxx_sb[:, k * G4 : (k + 1) * G4],
                                  Wxx_g[k * 128 : (k + 1) * 128, :])
                nc.vector.tensor_copy(Wxx_sb[:, k * G4 : (k + 1) * G4],
                                      Wxx_sb[:, k * G4 : (k + 1) * G4])
            for k in range(KY):
                nc.sync.dma_start(Wxy_sb[:, k * G4 : (k + 1) * G4],
                                  Wxy_g[k * 128 : (k + 1) * 128, :])
                nc.vector.tensor_copy(Wxy_sb[:, k * G4 : (k + 1) * G4],
                                      Wxy_sb[:, k * G4 : (k + 1) * G4])
            for k in range(KH):
                nc.sync.dma_start(Wh_sb[:, k * G4 : (k + 1) * G4],
                                  Wh_g[k * 128 : (k + 1) * 128, :])
                nc.vector.tensor_copy(Wh_sb[:, k * G4 : (k + 1) * G4],
                                      Wh_sb[:, k * G4 : (k + 1) * G4])
                nc.sync.dma_start(Wd_sb[:, k * O : (k + 1) * O],
                                  Wd_g[k * 128 : (k + 1) * 128, :])
                nc.vector.tensor_copy(Wd_sb[:, k * O : (k + 1) * O],
                                      Wd_sb[:, k * O : (k + 1) * O])
            if use_bias_z:
                bz_sb = wpool.tile([128, 1024], F32)
                nc.sync.dma_start(bz_sb[:], bz_d[:, :])
            if use_bias_y:
                by_sb = wpool.tile([BL, O], F32)
                nc.sync.dma_start(by_sb[:], by_d[:, :])

            # identity for PE transposes (16-bit: f32 transpose-mode faults on
            # hw); I64 in both partition halves so the fmap can start at
            # partition 0 or 64 (must match the weights)
            ident = wpool.tile([128, 128], mm_dt)
            make_identity(nc, ident[:])
            nc.vector.tensor_copy(ident[:], ident[:])  # launder Pool dep -> DVE

            def emit_z_mms(z_tile, chunks, start, stop):
                nck = len(chunks)
                for ci, (lhsT, wtile, coff) in enumerate(chunks):
                    for beta in range(2):
                        for j in range(4):
                            w_lo = coff + 2048 * beta + 512 * j
                            nc.tensor.matmul(
                                z_tile[32 * j : 32 * (j + 1),
                                       512 * beta : 512 * (beta + 1)],
                                mc(lhsT),
                                mc(wtile[:, w_lo : w_lo + 512]),
                                start=(start and ci == 0),
                                stop=(stop and ci == nck - 1),
                                tile_position=(0, 32 * j),
                                skip_group_check=True,
                            )

            def load_x(idx):
                """idx: python int or ScalarValue (dynamic) step index."""
                xw = 3 * BL if X12 else 2 * BL
                xT_sb = xpool.tile([128, xw], U8 if X12 else mm_dt,
                                   name="xT_sb")
                if isinstance(idx, int):
                    nc.sync.dma_start(xT_sb[:], xT_d[idx])
                else:
                    nc.sync.dma_start(xT_sb[:], xT_d[ds(idx, 1)].squeeze(0))
                xr_sb = xpool.tile([128, 2 * BL], mm_dt, name="xr_sb")
                if X12:
                    # unpack byte planes B0|B1|B2 -> integer v = u - 2048 in
                    # fp16 (exact for |v| <= 2047); these DVE ops also launder
                    # the x DMA-queue sem into the DVE sem
                    B0 = xT_sb[:, 0:BL]
                    B1 = xT_sb[:, BL : 2 * BL]
                    B2 = xT_sb[:, 2 * BL : 3 * BL]
                    b1c = xpool.tile([128, BL], mybir.dt.uint16, name="b1c")
                    ahi = xpool.tile([128, BL], mybir.dt.uint16, name="ahi")
                    alo = xpool.tile([128, BL], mybir.dt.uint16, name="alo")
                    # bitwise ops can't cast, so widen B1 via copy first;
                    # fused (op0, op1) pairs must also be same ALU class
                    nc.vector.tensor_copy(b1c[:], B1)
                    nc.vector.tensor_scalar(ahi[:], b1c[:], 0x0F, None,
                                            ALU.bitwise_and)
                    nc.vector.tensor_scalar(ahi[:], ahi[:], 256, None,
                                            ALU.mult)
                    nc.vector.tensor_copy(alo[:], B0)
                    nc.vector.tensor_add(ahi[:], ahi[:], alo[:])
                    nc.vector.tensor_scalar(
                        xr_sb[:, 0 : 2 * BL : 2], ahi[:], 2048, None,
                        ALU.subtract)
                    nc.vector.tensor_scalar(b1c[:], b1c[:], 4, None,
                                            ALU.logical_shift_right)
                    nc.vector.tensor_scalar(alo[:], B2, 16, None, ALU.mult)
                    nc.vector.tensor_add(b1c[:], b1c[:], alo[:])
                    nc.vector.tensor_scalar(
                        xr_sb[:, 1 : 2 * BL : 2], b1c[:], 2048, None,
                        ALU.subtract)
                else:
                    # launder the x DMA-queue sem into the DVE sem
                    nc.vector.tensor_copy(xr_sb[:], xT_sb[:])
                return [(xr_sb[:, bass.ts(k, BL)], Wxx_sb, k * G4)
                        for k in range(KX)]

            # Loop-carried feedback state must be FIXED tiles written in
            # place (like c_sb): per-iteration pool allocations read via a
            # pre-loop handle deadlock the tile scheduler at the back edge.
            # The uniform loop body always runs the h/y matmuls, so step 0
            # consumes the memset h_{-1}=y_{-1}=0 state.
            hT_state = hpool.tile([128, 256], mm_dt, name="hT_st")
            nc.gpsimd.memset(hT_state[:], 0.0)
            yT_state = ypool.tile([128, 2 * BL], mm_dt, name="yT_st")
            nc.gpsimd.memset(yT_state[:], 0.0)

            UNROLL = 8
            assert T_steps % UNROLL == 0

            # software pipeline: within a group, the x-part of step t+1 is
            # issued during step t, so the in-order PE has independent work
            # while the gate chain (ACT/DVE) of step t runs. The pipeline
            # restarts at each group boundary (the loop back-edge is a full
            # barrier), costing a few us per group.
            with tc.For_i(0, T_steps, UNROLL) as t0:
                z_ps = zpsum.tile([128, 1024], F32, name="z_ps")
                emit_z_mms(z_ps, load_x(t0), start=True, stop=False)
                for j in range(UNROLL):
                    # h first, y last: the y feedback chain (Wd+tanh+cast+
                    # transpose) of step t-1 gets the h-matmul span as slack
                    chunks = [(hT_state[:, _hT_off(k) : _hT_off(k) + BL],
                               Wh_sb, k * G4) for k in (0, 2, 4, 6, 1, 3, 5, 7)]
                    chunks += [(yT_state[:, bass.ts(k, BL)], Wxy_sb, k * G4)
                               for k in range(KY)]
                    emit_z_mms(z_ps, chunks, start=False, stop=True)
                    if j + 1 < UNROLL:
                        z_next = zpsum.tile([128, 1024], F32, name="z_ps")
                        emit_z_mms(z_next, load_x(t0 + (j + 1)), start=True,
                                   stop=False)
                    else:
                        z_next = None

                    # gate math: <=1 PSUM operand per DVE op
                    if use_bias_z:
                        nc.vector.tensor_add(z_ps[:, 0:512], z_ps[:, 0:512],
                                             bz_sb[:, 0:512])
                        nc.vector.tensor_add(z_ps[:, 512:1024],
                                             z_ps[:, 512:1024],
                                             bz_sb[:, 512:1024])
                    # gate chain split into column halves: half 0 finishes ->
                    # its transpose + hT copy run while half 1 still computes,
                    # so the even hT-chunk matmuls of step t+1 start earlier
                    tg_sb = gpool.tile([128, 256], F32, name="tg_sb")
                    o_sb = gpool.tile([128, 256], F32, name="o_sb")
                    h_stk = gpool.tile([128, 256], mm_dt, name="h_stk")
                    tr_ps = tpsum.tile([128, 320], mm_dt, name="tr_ps")
                    hT_sb = hT_state
                    for hf in range(2):
                        s = slice(128 * hf, 128 * hf + 128)
                        nc.scalar.activation(tg_sb[:, s],
                                             z_ps[:, 512 + 128 * hf :
                                                  640 + 128 * hf],
                                             AF.Tanh)
                        nc.scalar.activation(z_ps[:, s], z_ps[:, s], AF.Sigmoid)
                        nc.vector.tensor_mul(tg_sb[:, s], z_ps[:, s],
                                             tg_sb[:, s])
                        nc.scalar.activation(z_ps[:, 256 + 128 * hf :
                                                  384 + 128 * hf],
                                             z_ps[:, 256 + 128 * hf :
                                                  384 + 128 * hf],
                                             AF.Sigmoid)
                        nc.vector.tensor_mul(c_sb[:, s],
                                             z_ps[:, 256 + 128 * hf :
                                                  384 + 128 * hf],
                                             c_sb[:, s])
                        nc.scalar.activation(o_sb[:, s],
                                             z_ps[:, 768 + 128 * hf :
                                                  896 + 128 * hf],
                                             AF.Sigmoid)
                        nc.vector.tensor_add(c_sb[:, s], tg_sb[:, s],
                                             c_sb[:, s])
                        nc.scalar.activation(tg_sb[:, s], c_sb[:, s], AF.Tanh)
                        nc.vector.tensor_mul(h_stk[:, s], o_sb[:, s],
                                             tg_sb[:, s])
                        nc.tensor.transpose(tr_ps[:, s], h_stk[:, s], ident[:])
                        nc.vector.tensor_copy(hT_sb[:, s], tr_ps[:, s])

                    # y = tanh(h @ Wd + bd)
                    y_ps = ypsum.tile([BL, O], F32, name="y_ps")
                    for k in range(KH):
                        nc.tensor.matmul(
                            y_ps[:],
                            mc(hT_sb[:, _hT_off(k) : _hT_off(k) + BL]),
                            mc(Wd_sb[:, k * O : (k + 1) * O]),
                            start=(k == 0),
                            stop=(k == KH - 1),
                        )
                    if use_bias_y:
                        nc.vector.tensor_add(y_ps[:], y_ps[:], by_sb[:])
                    y_sb = ypool.tile([BL, O], F32, name="y_sb", bufs=2)
                    nc.scalar.activation(y_sb[:], y_ps[:], AF.Tanh)
                    # cast y for the fp16 PE-transposes (also launders
                    # ACT -> DVE); this is on the feedback critical path, so
                    # it runs before the u8 output quantization
                    y_bf = ypool.tile([BL, O], mm_dt, name="y_bf")
                    nc.vector.tensor_copy(y_bf[:], y_sb[:])
                    # own double-buffered tile so the output DMA never blocks
                    # the next step's gate ACTs
                    if out_u8:
                        y_out = ypool.tile([BL, O], U8, name="y_out", bufs=2)
                        nc.vector.tensor_scalar(y_out[:], y_sb[:], 127.0,
                                                u8_bias, ALU.mult, ALU.add)
                    else:
                        y_out = ypool.tile([BL, O], mm_dt, name="y_out",
                                           bufs=2)
                        nc.vector.tensor_copy(y_out[:], y_sb[:])
                    nc.sync.dma_start(ys_d[ds(t0 + j, 1)].squeeze(0), y_out[:])

                    # y -> yT via 2 PE transposes
                    for q in range(2):
                        nc.tensor.transpose(
                            tr_ps[:, 256 + 32 * q : 256 + 32 * (q + 1)],
                            y_bf[0:BL, 128 * q : 128 * (q + 1)],
                            ident[0:32, 0:32],
                        )
                    nc.vector.tensor_copy(yT_state[:], tr_ps[:, 256:320])

                    z_ps = z_next

    nc.compile()
    return nc


def prep_inputs(x, Wx, Wh, b, Wd, bd, T_steps: int = T,
                mm_np=np.float16):
    """Host-side shard + relayout. Returns (in_maps, use_bias_z, use_bias_y)."""
    x = np.asarray(x, dtype=np.float32)[:, :T_steps, :]
    Wx = np.asarray(Wx, dtype=np.float32)
    Wh = np.asarray(Wh, dtype=np.float32)
    b = np.asarray(b, dtype=np.float32)
    Wd = np.asarray(Wd, dtype=np.float32)
    bd = np.asarray(bd, dtype=np.float32)

    perm = gate_perm()
    if X12:
        xs = float(np.abs(x).max()) / 2047.0  # folded into Wxx below
    else:
        xs = 1.0
    Wxp = Wx[:, perm]
    Wxx = np.ascontiguousarray(Wxp[:D] * xs).astype(mm_np)
    Wxy = np.ascontiguousarray(Wxp[D:]).astype(mm_np)
    Whp = np.ascontiguousarray(Wh[:, perm]).astype(mm_np)
    Wd = Wd.astype(mm_np)

    use_bias_z = bool(np.any(b))
    use_bias_y = bool(np.any(bd))
    shared = {}
    if use_bias_z:
        bp = b[perm]
        bz = np.empty((128, 1024), dtype=np.float32)
        for j in range(4):
            for beta in range(2):
                bz[32 * j : 32 * (j + 1), 512 * beta : 512 * (beta + 1)] = bp[
                    2048 * beta + 512 * j : 2048 * beta + 512 * j + 512][None, :]
        shared["bz"] = bz
    if use_bias_y:
        shared["by"] = np.broadcast_to(bd, (BL, O)).copy()

    if X12:
        xu = (np.round(x / xs).astype(np.int32) + 2048).astype(np.uint16)
    in_maps = []
    for c in range(NCORES):
        if X12:
            xc = xu[c * BL : (c + 1) * BL]                 # [BL, T, D] u16
        else:
            xc = x[c * BL : (c + 1) * BL]
        xT = xc.transpose(1, 2, 0)                         # [T, D, BL]
        xT = xT.reshape(T_steps, 2, 128, BL).transpose(0, 2, 1, 3)
        xT = xT.reshape(T_steps, 128, 2 * BL)
        if X12:
            a = xT[:, :, 0::2].astype(np.uint16)           # [T, 128, BL]
            bb = xT[:, :, 1::2].astype(np.uint16)
            B0 = (a & 0xFF).astype(np.uint8)
            B1 = ((a >> 8) | ((bb & 0xF) << 4)).astype(np.uint8)
            B2 = (bb >> 4).astype(np.uint8)
            xT = np.ascontiguousarray(
                np.concatenate([B0, B1, B2], axis=2))      # [T, 128, 3*BL]
        else:
            xT = np.ascontiguousarray(xT).astype(mm_np)
        wsh = np.concatenate([
            Wxx[c * WXS : (c + 1) * WXS].ravel(),
            Wxy[c * WXS : (c + 1) * WXS].ravel(),
            Whp[c * WHS : (c + 1) * WHS].ravel(),
            Wd[c * WHS : (c + 1) * WHS].ravel(),
        ])
        in_maps.append({"xT": xT, "wsh": wsh, **shared})
    return in_maps, use_bias_z, use_bias_y


def decode_ys(res, out_u8: bool = True):
    """Concatenate per-core results and decode to fp32 [B, T, O]."""
    parts = []
    for c in range(NCORES):
        ys = res.results[c]["ys"]                 # [T, BL, O] t-major
        ys = np.ascontiguousarray(ys.transpose(1, 0, 2))  # [BL, T, O]
        if out_u8:
            parts.append((ys.astype(np.float32) - 128.0) * (1.0 / 127.0))
        else:
            parts.append(ys.astype(np.float32))
    return np.concatenate(parts, axis=0)


def kernel(x, Wx, Wh, b, Wd, bd):
    from concourse.bass_utils import run_bass_kernel_spmd

    in_maps, ubz, uby = prep_inputs(x, Wx, Wh, b, Wd, bd, T)
    nc = build_nc(T, ubz, uby)
    res = run_bass_kernel_spmd(nc, in_maps, list(range(NCORES)))
    return decode_ys(res)


# revision 26
# speedup vs baseline: 5.5716x; 1.0790x over previous
"""Autoregressive LSTM cell (B=256, T=256, D=256, H=1024, O=256) on 8 TRN2 cores.

Strategy: pure data-parallel over batch (32 rows/core). The end-to-end time is
dominated by (a) host<->device transfer over the axon tunnel (~60-70MB/s h2d,
~50MB/s d2h) and (b) per-call jit re-trace/lowering that scales with program
size -- NOT device compute (~2ms). Three structural choices follow:
  - Shipped bytes are minimized: x goes as 12-bit fixed point packed into
    byte planes (3MB/core; u = round(x/s)+2048, the scale s is folded into
    the host-side Wxx so the device just subtracts 2048); weights go SHARDED
    1/8 per core in one flat fp16 tensor (~1.6MB/core) and are reassembled
    on device with four DRAM AllGather collectives; the output goes uint8
    (u = round(127*y)+128, decoded host-side as (u-128)/127; the DVE
    float->u8 convert rounds-to-nearest, measured on hw).
  - The 256 timesteps run under a hardware For_i loop (8 steps per
    iteration), so the program is ~2.3k instructions instead of ~64k; build,
    BIR serialization, jit lowering, NEFF compile and model load all shrink
    accordingly (the lowering happens on every run_bass_kernel_spmd call).
  - The matmul/feedback pipeline runs fp16 (vs bf16), dropping the
    fp32-reference error to ~9e-4 so the x/output quantization fits easily:
    total measured error 5.8e-3 vs the 2e-2 gate.

Per step t (sequential, 256 steps):
    z = x_t @ Wxx + y_{t-1} @ Wxy + h_{t-1} @ Wh     (+b)
    i,f,g,o gates -> c = sig(f)*c + sig(i)*tanh(g); h = sig(o)*tanh(c)
    y = tanh(h @ Wd + bd)
Matmuls are "activation-stationary": lhsT = activation^T [K<=128, M=32batch],
rhs streams fp16 weight columns at 1 column/cycle (fp32 PSUM accumulation;
fp32 gate math). 4-way PE column tiling (tile_position col groups) packs 4
batch-32 matmuls concurrently, writing z in a stacked PSUM layout:
  z_ps [128, 1024]: position (32j+b, 512*beta + n) = z_perm[2048*beta + 512j + n]
Weight columns are host-permuted so that
  bank0 (cols 0:512)  = [ sig-gate i | sig-gate f ] column-paired per channel
  bank1 (cols 512:1024) = [ tanh-gate g | sig-gate o ]
with channel(p=32j+b, n) = 256j + (n mod 256): all gate elementwise ops are
partition-aligned and the c update is a free-dim-shifted add.
h/y are fed back transposed via PE transpose-mode matmuls.

Overlap structure: the x-part matmuls of step t+1 are software-pipelined into
step t, z-chunks are ordered x->h->y so the y-feedback chain hides under the
h-matmul span, and the gate chain runs in two column halves. The pipeline
restarts at each loop back-edge (a full barrier); loop-carried state (h^T,
y^T, c) lives in fixed SBUF tiles written in place.

Measured (axon tunnel, cached-compile re-run incl. transfers): ~1.25s
end-to-end for the full call, vs ~8.1s for the session-start baseline
(fully unrolled bf16 kernel shipping replicated fp32-I/O tensors).
"""

import sys

for p in ("/opt/trn_rl_repo",):
    if p not in sys.path:
        sys.path.insert(0, p)

from contextlib import ExitStack

import numpy as np

import concourse.bacc as bacc
import concourse.bass as bass
import concourse.mybir as mybir
import concourse.tile as tile
from concourse.bass import ds
from concourse.masks import make_identity

F32 = mybir.dt.float32
U8 = mybir.dt.uint8
AF = mybir.ActivationFunctionType
ALU = mybir.AluOpType

B, T, D, H, O = 256, 256, 256, 1024, 256
NCORES = 8
BL = B // NCORES  # 32
G4 = 4 * H  # 4096
KX, KY, KH = D // 128, O // 128, H // 128  # 2, 2, 8
WXS = D // NCORES  # 32 weight-shard rows per core for Wxx/Wxy
WHS = H // NCORES  # 128 shard rows for Wh/Wd
# flat per-core weight shard: [Wxx | Wxy | Wh | Wd] row-shards, one tensor so
# the tunnel pays one per-array transfer latency instead of four
WOFF = (0, WXS * G4, 2 * WXS * G4, 2 * WXS * G4 + WHS * G4,
        2 * WXS * G4 + WHS * G4 + WHS * O)
WSH_N = WOFF[4]  # 819200
# x ships as 12-bit fixed point: u = round(x/s) + 2048, s = max|x|/2047,
# packed as byte planes [B0 | B1 | B2] (2 values -> 3 bytes). The device
# unpacks to the integer v = u - 2048 in fp16; s is folded into Wxx host-side.
X12 = True

# u8 output encoding: u = convert_u8(127*y + U8_BIAS), decoded (u-128)/127.
# The DVE float->u8 convert rounds-to-nearest (measured on hw: mean code
# offset -0.001, std 0.289), so the bias is exactly 128.0.
U8_BIAS_DEFAULT = 128.0


def gate_perm() -> np.ndarray:
    """Map stored z column position -> original gate column (i,f,g,o order)."""
    perm = np.empty(G4, dtype=np.int64)
    for beta in (0, 1):
        for j in range(4):
            for half in (0, 1):
                gate = (0, 1, 2, 3)[2 * beta + half]
                src = 1024 * gate + 256 * j
                pos = 2048 * beta + 512 * j + 256 * half
                perm[pos : pos + 256] = np.arange(src, src + 256)
    return perm


def _hT_off(c: int) -> int:
    """Column offset of h^T chunk c (channels 128c:128c+128) inside hT_sb."""
    return 128 * (c % 2) + 32 * (c // 2)


def build_nc(T_steps: int = T, use_bias_z: bool = False, use_bias_y: bool = False,
             mm_dt=mybir.dt.float16, out_u8: bool = True,
             u8_bias: float = U8_BIAS_DEFAULT):
    nc = bacc.Bacc()

    if X12:
        xT_d = nc.declare_dram_parameter("xT", [T_steps, 128, 3 * BL], U8,
                                         isOutput=False)
    else:
        xT_d = nc.declare_dram_parameter("xT", [T_steps, 128, 2 * BL], mm_dt,
                                         isOutput=False)
    # weights arrive as one flat row-shard: core c holds rows
    # [c*WXS:(c+1)*WXS] of Wxx/Wxy and rows [c*WHS:(c+1)*WHS] of Wh/Wd;
    # AllGather rebuilds the full matrices in DRAM (saves 7/8 of the weight
    # bytes over the tunnel).
    wsh_d = nc.declare_dram_parameter("wsh", [WSH_N], mm_dt, isOutput=False)
    bz_d = by_d = None
    if use_bias_z:
        bz_d = nc.declare_dram_parameter("bz", [128, 1024], F32, isOutput=False)
    if use_bias_y:
        by_d = nc.declare_dram_parameter("by", [BL, O], F32, isOutput=False)
    out_dt = U8 if out_u8 else mm_dt
    # t-major so the per-step store is one outer-dim (dynamic) slice
    ys_d = nc.declare_dram_parameter("ys", [T_steps, BL, O], out_dt,
                                     isOutput=True)

    def mc(ap):
        return ap.bitcast(mm_dt) if ap.dtype != mm_dt else ap

    with tile.TileContext(nc) as tc:
        with ExitStack() as ctx:
            dpool = ctx.enter_context(
                tc.tile_pool(name="dramw", bufs=1, space="DRAM"))
            wpool = ctx.enter_context(tc.tile_pool(name="weights", bufs=1))
            state = ctx.enter_context(tc.tile_pool(name="state", bufs=1))
            xpool = ctx.enter_context(tc.tile_pool(name="xin", bufs=3))
            gpool = ctx.enter_context(tc.tile_pool(name="gates", bufs=1))
            hpool = ctx.enter_context(tc.tile_pool(name="hT", bufs=1))
            ypool = ctx.enter_context(tc.tile_pool(name="yt", bufs=1))
            zpsum = ctx.enter_context(tc.tile_pool(name="zps", bufs=2, space="PSUM"))
            ypsum = ctx.enter_context(tc.tile_pool(name="yps", bufs=2, space="PSUM"))
            tpsum = ctx.enter_context(tc.tile_pool(name="tps", bufs=2, space="PSUM"))

            # c state, channel(32j+b, n) = 256j + n  (memset first: needed at
            # t=0, and it shares the gpsimd queue with the gathers below)
            c_sb = state.tile([128, 256], F32)
            nc.gpsimd.memset(c_sb[:], 0.0)

            # ---- on-device weight reassembly: DRAM AllGather per matrix ----
            # bounce buffer: collectives can't operate on I/O tensors. One
            # bounce DMA, then four gathers reading flat slices of it; each
            # gather's output lands in the matrix's natural row-major layout
            # (rank r's rows land at block r).
            wsh_b = dpool.tile([WSH_N], mm_dt, name="wsh_b")
            nc.gpsimd.dma_start(wsh_b[:], wsh_d[:])
            Wxx_g = dpool.tile([D, G4], mm_dt, name="Wxx_g")
            Wxy_g = dpool.tile([O, G4], mm_dt, name="Wxy_g")
            Wh_g = dpool.tile([H, G4], mm_dt, name="Wh_g")
            Wd_g = dpool.tile([H, O], mm_dt, name="Wd_g")
            RG = [list(range(NCORES))]
            # gather order = first-use order: Wxx (t=0 z), Wd (t=0 y), Wh/Wxy
            # (t=1 z)
            for (a, b), g in (((WOFF[0], WOFF[1]), Wxx_g),
                              ((WOFF[3], WOFF[4]), Wd_g),
                              ((WOFF[2], WOFF[3]), Wh_g),
                              ((WOFF[1], WOFF[2]), Wxy_g)):
                nc.gpsimd.collective_compute(
                    "AllGather", ALU.bypass, replica_groups=RG,
                    ins=[wsh_b[a:b].opt()], outs=[g.opt()])

            Wxx_sb = wpool.tile([128, KX * G4], mm_dt)
            Wxy_sb = wpool.tile([128, KY * G4], mm_dt)
            Wh_sb = wpool.tile([128, KH * G4], mm_dt)
            Wd_sb = wpool.tile([128, KH * O], mm_dt)
            # Matmult instructions can carry at most ONE sem wait in this
            # lowering; every matmul dependency must resolve to a single DVE
            # sem value. Weight DMAs are therefore "laundered" through
            # in-place DVE copies (one per DMA so each copy waits on one
            # DMA-queue sem only).
            for k in range(KX):
                nc.sync.dma_start(Wxx_sb[:, k * G4 : (k + 1) * G4],
                                  Wxx_g[k * 128 : (k + 1) * 128, :])
                nc.vector.tensor_copy(Wxx_sb[:, k * G4 : (k + 1) * G4],
                                      Wxx_sb[:, k * G4 : (k + 1) * G4])
            for k in range(KY):
                nc.sync.dma_start(Wxy_sb[:, k * G4 : (k + 1) * G4],
                                  Wxy_g[k * 128 : (k + 1) * 128, :])
                nc.vector.tensor_copy(Wxy_sb[:, k * G4 : (k + 1) * G4],
                                      Wxy_sb[:, k * G4 : (k + 1) * G4])
            for k in range(KH):
                nc.sync.dma_start(Wh_sb[:, k * G4 : (k + 1) * G4],
                                  Wh_g[k * 128 : (k + 1) * 128, :])
                nc.vector.tensor_copy(Wh_sb[:, k * G4 : (k + 1) * G4],
                                      Wh_sb[:, k * G4 : (k + 1) * G4])
                nc.sync.dma_start(Wd_sb[:, k * O : (k + 1) * O],
                                  Wd_g[k * 128 : (k + 1) * 128, :])
                nc.vector.tensor_copy(Wd_sb[:, k * O : (k + 1) * O],
                                      Wd_sb[:, k * O : (k + 1) * O])
            if use_bias_z:
                bz_sb = wpool.tile([128, 1024], F32)
                nc.sync.dma_start(bz_sb[:], bz_d[:, :])
            if use_bias_y:
                by_sb = wpool.tile([BL, O], F32)
                nc.sync.dma_start(by_sb[:], by_d[:, :])

            # identity for PE transposes (16-bit: f32 transpose-mode faults on
            # hw); I64 in both partition halves so the fmap can start at
            # partition 0 or 64 (must match the weights)
            ident = wpool.tile([128, 128], mm_dt)
            make_identity(nc, ident[:])
            nc.vector.tensor_copy(ident[:], ident[:])  # launder Pool dep -> DVE

            def emit_z_mms(z_tile, chunks, start, stop):
                nck = len(chunks)
                for ci, (lhsT, wtile, coff) in enumerate(chunks):
                    for beta in range(2):
                        for j in range(4):
                            w_lo = coff + 2048 * beta + 512 * j
                            nc.tensor.matmul(
                                z_tile[32 * j : 32 * (j + 1),
                                       512 * beta : 512 * (beta + 1)],
                                mc(lhsT),
                                mc(wtile[:, w_lo : w_lo + 512]),
                                start=(start and ci == 0),
                                stop=(stop and ci == nck - 1),
                                tile_position=(0, 32 * j),
                                skip_group_check=True,
                            )

            def load_x(idx):
                """idx: python int or ScalarValue (dynamic) step index."""
                xw = 3 * BL if X12 else 2 * BL
                xT_sb = xpool.tile([128, xw], U8 if X12 else mm_dt,
                                   name="xT_sb")
                if isinstance(idx, int):
                    nc.sync.dma_start(xT_sb[:], xT_d[idx])
                else:
                    nc.sync.dma_start(xT_sb[:], xT_d[ds(idx, 1)].squeeze(0))
                xr_sb = xpool.tile([128, 2 * BL], mm_dt, name="xr_sb")
                if X12:
                    # unpack byte planes B0|B1|B2 -> integer v = u - 2048 in
                    # fp16 (exact for |v| <= 2047); these DVE ops also launder
                    # the x DMA-queue sem into the DVE sem
                    B0 = xT_sb[:, 0:BL]
                    B1 = xT_sb[:, BL : 2 * BL]
                    B2 = xT_sb[:, 2 * BL : 3 * BL]
                    b1c = xpool.tile([128, BL], mybir.dt.uint16, name="b1c")
                    ahi = xpool.tile([128, BL], mybir.dt.uint16, name="ahi")
                    alo = xpool.tile([128, BL], mybir.dt.uint16, name="alo")
                    # bitwise ops can't cast, so widen B1 via copy first;
                    # fused (op0, op1) pairs must also be same ALU class
                    nc.vector.tensor_copy(b1c[:], B1)
                    nc.vector.tensor_scalar(ahi[:], b1c[:], 0x0F, None,
                                            ALU.bitwise_and)
                    nc.vector.tensor_scalar(ahi[:], ahi[:], 256, None,
                                            ALU.mult)
                    nc.vector.tensor_copy(alo[:], B0)
                    nc.vector.tensor_add(ahi[:], ahi[:], alo[:])
                    nc.vector.tensor_scalar(
                        xr_sb[:, 0 : 2 * BL : 2], ahi[:], 2048, None,
                        ALU.subtract)
                    nc.vector.tensor_scalar(b1c[:], b1c[:], 4, None,
                                            ALU.logical_shift_right)
                    nc.vector.tensor_scalar(alo[:], B2, 16, None, ALU.mult)
                    nc.vector.tensor_add(b1c[:], b1c[:], alo[:])
                    nc.vector.tensor_scalar(
                        xr_sb[:, 1 : 2 * BL : 2], b1c[:], 2048, None,
                        ALU.subtract)
                else:
                    # launder the x DMA-queue sem into the DVE sem
                    nc.vector.tensor_copy(xr_sb[:], xT_sb[:])
                return [(xr_sb[:, bass.ts(k, BL)], Wxx_sb, k * G4)
                        for k in range(KX)]

            # Loop-carried feedback state must be FIXED tiles written in
            # place (like c_sb): per-iteration pool allocations read via a
            # pre-loop handle deadlock the tile scheduler at the back edge.
            # The uniform loop body always runs the h/y matmuls, so step 0
            # consumes the memset h_{-1}=y_{-1}=0 state.
            hT_state = hpool.tile([128, 256], mm_dt, name="hT_st")
            nc.gpsimd.memset(hT_state[:], 0.0)
            yT_state = ypool.tile([128, 2 * BL], mm_dt, name="yT_st")
            nc.gpsimd.memset(yT_state[:], 0.0)

            UNROLL = 8
            assert T_steps % UNROLL == 0

            # software pipeline: within a group, the x-part of step t+1 is
            # issued during step t, so the in-order PE has independent work
            # while the gate chain (ACT/DVE) of step t runs. The pipeline
            # restarts at each group boundary (the loop back-edge is a full
            # barrier), costing a few us per group.
            with tc.For_i(0, T_steps, UNROLL) as t0:
                z_ps = zpsum.tile([128, 1024], F32, name="z_ps")
                emit_z_mms(z_ps, load_x(t0), start=True, stop=False)
                for j in range(UNROLL):
                    # h first, y last: the y feedback chain (Wd+tanh+cast+
                    # transpose) of step t-1 gets the h-matmul span as slack
                    chunks = [(hT_state[:, _hT_off(k) : _hT_off(k) + BL],
                               Wh_sb, k * G4) for k in (0, 2, 4, 6, 1, 3, 5, 7)]
                    chunks += [(yT_state[:, bass.ts(k, BL)], Wxy_sb, k * G4)
                               for k in range(KY)]
                    emit_z_mms(z_ps, chunks, start=False, stop=True)
                    if j + 1 < UNROLL:
                        z_next = zpsum.tile([128, 1024], F32, name="z_ps")
                        emit_z_mms(z_next, load_x(t0 + (j + 1)), start=True,
                                   stop=False)
                    else:
                        z_next = None

                    # gate math: <=1 PSUM operand per DVE op
                    if use_bias_z:
                        nc.vector.tensor_add(z_ps[:, 0:512], z_ps[:, 0:512],
                                             bz_sb[:, 0:512])
                        nc.vector.tensor_add(z_ps[:, 512:1024],
                                             z_ps[:, 512:1024],
                                             bz_sb[:, 512:1024])
                    # gate chain split into column halves: half 0 finishes ->
                    # its transpose + hT copy run while half 1 still computes,
                    # so the even hT-chunk matmuls of step t+1 start earlier
                    tg_sb = gpool.tile([128, 256], F32, name="tg_sb")
                    o_sb = gpool.tile([128, 256], F32, name="o_sb")
                    h_stk = gpool.tile([128, 256], mm_dt, name="h_stk")
                    tr_ps = tpsum.tile([128, 320], mm_dt, name="tr_ps")
                    hT_sb = hT_state
                    for hf in range(2):
                        s = slice(128 * hf, 128 * hf + 128)
                        nc.scalar.activation(tg_sb[:, s],
                                             z_ps[:, 512 + 128 * hf :
                                                  640 + 128 * hf],
                                             AF.Tanh)
                        nc.scalar.activation(z_ps[:, s], z_ps[:, s], AF.Sigmoid)
                        nc.vector.tensor_mul(tg_sb[:, s], z_ps[:, s],
                                             tg_sb[:, s])
                        nc.scalar.activation(z_ps[:, 256 + 128 * hf :
                                                  384 + 128 * hf],
                                             z_ps[:, 256 + 128 * hf :
                                                  384 + 128 * hf],
                                             AF.Sigmoid)
                        nc.vector.tensor_mul(c_sb[:, s],
                                             z_ps[:, 256 + 128 * hf :
                                                  384 + 128 * hf],
                                             c_sb[:, s])
                        nc.scalar.activation(o_sb[:, s],
                                             z_ps[:, 768 + 128 * hf :
                                                  896 + 128 * hf],
                                             AF.Sigmoid)
                        nc.vector.tensor_add(c_sb[:, s], tg_sb[:, s],
                                             c_sb[:, s])
                        nc.scalar.activation(tg_sb[:, s], c_sb[:, s], AF.Tanh)
                        nc.vector.tensor_mul(h_stk[:, s], o_sb[:, s],
                                             tg_sb[:, s])
                        nc.tensor.transpose(tr_ps[:, s], h_stk[:, s], ident[:])
                        nc.vector.tensor_copy(hT_sb[:, s], tr_ps[:, s])

                    # y = tanh(h @ Wd + bd)
                    y_ps = ypsum.tile([BL, O], F32, name="y_ps")
                    for k in range(KH):
                        nc.tensor.matmul(
                            y_ps[:],
                            mc(hT_sb[:, _hT_off(k) : _hT_off(k) + BL]),
                            mc(Wd_sb[:, k * O : (k + 1) * O]),
                            start=(k == 0),
                            stop=(k == KH - 1),
                        )
                    if use_bias_y:
                        nc.vector.tensor_add(y_ps[:], y_ps[:], by_sb[:])
                    y_sb = ypool.tile([BL, O], F32, name="y_sb", bufs=2)
                    nc.scalar.activation(y_sb[:], y_ps[:], AF.Tanh)
                    # cast y for the fp16 PE-transposes (also launders
                    # ACT -> DVE); this is on the feedback critical path, so
                    # it runs before the u8 output quantization
                    y_bf = ypool.tile([BL, O], mm_dt, name="y_bf")
                    nc.vector.tensor_copy(y_bf[:], y_sb[:])
                    # own double-buffered tile so the output DMA never blocks
                    # the next step's gate ACTs
                    if out_u8:
                        y_out = ypool.tile([BL, O], U8, name="y_out", bufs=2)
                        nc.vector.tensor_scalar(y_out[:], y_sb[:], 127.0,
                                                u8_bias, ALU.mult, ALU.add)
                    else:
                        y_out = ypool.tile([BL, O], mm_dt, name="y_out",
                                           bufs=2)
                        nc.vector.tensor_copy(y_out[:], y_sb[:])
                    nc.sync.dma_start(ys_d[ds(t0 + j, 1)].squeeze(0), y_out[:])

                    # y -> yT via 2 PE transposes
                    for q in range(2):
                        nc.tensor.transpose(
                            tr_ps[:, 256 + 32 * q : 256 + 32 * (q + 1)],
                            y_bf[0:BL, 128 * q : 128 * (q + 1)],
                            ident[0:32, 0:32],
                        )
                    nc.vector.tensor_copy(yT_state[:], tr_ps[:, 256:320])

                    z_ps = z_next

    nc.compile()
    return nc


def prep_inputs(x, Wx, Wh, b, Wd, bd, T_steps: int = T,
                mm_np=np.float16):
    """Host-side shard + relayout. Returns (in_maps, use_bias_z, use_bias_y)."""
    x = np.asarray(x, dtype=np.float32)[:, :T_steps, :]
    Wx = np.asarray(Wx, dtype=np.float32)
    Wh = np.asarray(Wh, dtype=np.float32)
    b = np.asarray(b, dtype=np.float32)
    Wd = np.asarray(Wd, dtype=np.float32)
    bd = np.asarray(bd, dtype=np.float32)

    perm = gate_perm()
    if X12:
        xs = max(float(np.abs(x).max()), 1e-20) / 2047.0  # folded into Wxx
    else:
        xs = 1.0
    Wxp = Wx[:, perm]
    Wxx = np.ascontiguousarray(Wxp[:D] * xs).astype(mm_np)
    Wxy = np.ascontiguousarray(Wxp[D:]).astype(mm_np)
    Whp = np.ascontiguousarray(Wh[:, perm]).astype(mm_np)
    Wd = Wd.astype(mm_np)

    use_bias_z = bool(np.any(b))
    use_bias_y = bool(np.any(bd))
    shared = {}
    if use_bias_z:
        bp = b[perm]
        bz = np.empty((128, 1024), dtype=np.float32)
        for j in range(4):
            for beta in range(2):
                bz[32 * j : 32 * (j + 1), 512 * beta : 512 * (beta + 1)] = bp[
                    2048 * beta + 512 * j : 2048 * beta + 512 * j + 512][None, :]
        shared["bz"] = bz
    if use_bias_y:
        shared["by"] = np.broadcast_to(bd, (BL, O)).copy()

    if X12:
        xu = (np.round(x / xs).astype(np.int32) + 2048).astype(np.uint16)
    in_maps = []
    for c in range(NCORES):
        if X12:
            xc = xu[c * BL : (c + 1) * BL]                 # [BL, T, D] u16
        else:
            xc = x[c * BL : (c + 1) * BL]
        xT = xc.transpose(1, 2, 0)                         # [T, D, BL]
        xT = xT.reshape(T_steps, 2, 128, BL).transpose(0, 2, 1, 3)
        xT = xT.reshape(T_steps, 128, 2 * BL)
        if X12:
            a = xT[:, :, 0::2].astype(np.uint16)           # [T, 128, BL]
            bb = xT[:, :, 1::2].astype(np.uint16)
            B0 = (a & 0xFF).astype(np.uint8)
            B1 = ((a >> 8) | ((bb & 0xF) << 4)).astype(np.uint8)
            B2 = (bb >> 4).astype(np.uint8)
            xT = np.ascontiguousarray(
                np.concatenate([B0, B1, B2], axis=2))      # [T, 128, 3*BL]
        else:
            xT = np.ascontiguousarray(xT).astype(mm_np)
        wsh = np.concatenate([
            Wxx[c * WXS : (c + 1) * WXS].ravel(),
            Wxy[c * WXS : (c + 1) * WXS].ravel(),
            Whp[c * WHS : (c + 1) * WHS].ravel(),
            Wd[c * WHS : (c + 1) * WHS].ravel(),
        ])
        in_maps.append({"xT": xT, "wsh": wsh, **shared})
    return in_maps, use_bias_z, use_bias_y


def decode_ys(res, out_u8: bool = True):
    """Concatenate per-core results and decode to fp32 [B, T, O]."""
    parts = []
    for c in range(NCORES):
        ys = res.results[c]["ys"]                 # [T, BL, O] t-major
        ys = np.ascontiguousarray(ys.transpose(1, 0, 2))  # [BL, T, O]
        if out_u8:
            parts.append((ys.astype(np.float32) - 128.0) * (1.0 / 127.0))
        else:
            parts.append(ys.astype(np.float32))
    return np.concatenate(parts, axis=0)


_NC_CACHE = {}


def kernel(x, Wx, Wh, b, Wd, bd):
    from concourse.bass_utils import run_bass_kernel_spmd

    in_maps, ubz, uby = prep_inputs(x, Wx, Wh, b, Wd, bd, T)
    key = (T, ubz, uby)
    nc = _NC_CACHE.get(key)
    if nc is None:
        nc = _NC_CACHE[key] = build_nc(T, ubz, uby)
    res = run_bass_kernel_spmd(nc, in_maps, list(range(NCORES)))
    return decode_ys(res)


# revision 32
# speedup vs baseline: 5.9756x; 1.0725x over previous
"""Autoregressive LSTM cell (B=256, T=256, D=256, H=1024, O=256) on 8 TRN2 cores.

Strategy: pure data-parallel over batch (32 rows/core). The end-to-end time is
dominated by (a) host<->device transfer over the axon tunnel (~60-70MB/s h2d,
~50MB/s d2h) and (b) per-call jit re-trace/lowering that scales with program
size -- NOT device compute (~2ms). Three structural choices follow:
  - Shipped bytes are minimized: x goes as 12-bit fixed point packed into
    byte planes (3MB/core; u = round(x/s)+2048, the scale s is folded into
    the host-side Wxx so the device just subtracts 2048); weights go SHARDED
    1/8 per core in one flat fp16 tensor (~1.6MB/core) and are reassembled
    on device with four DRAM AllGather collectives; the output goes uint8
    (u = round(127*y)+128, decoded host-side as (u-128)/127; the DVE
    float->u8 convert rounds-to-nearest, measured on hw).
  - The 256 timesteps run under a hardware For_i loop (8 steps per
    iteration), so the program is ~2.3k instructions instead of ~64k; build,
    BIR serialization, jit lowering, NEFF compile and model load all shrink
    accordingly (the lowering happens on every run_bass_kernel_spmd call).
  - The matmul/feedback pipeline runs fp16 (vs bf16), dropping the
    fp32-reference error to ~9e-4 so the x/output quantization fits easily:
    total measured error 5.8e-3 vs the 2e-2 gate.

Per step t (sequential, 256 steps):
    z = x_t @ Wxx + y_{t-1} @ Wxy + h_{t-1} @ Wh     (+b)
    i,f,g,o gates -> c = sig(f)*c + sig(i)*tanh(g); h = sig(o)*tanh(c)
    y = tanh(h @ Wd + bd)
Matmuls are "activation-stationary": lhsT = activation^T [K<=128, M=32batch],
rhs streams fp16 weight columns at 1 column/cycle (fp32 PSUM accumulation;
fp32 gate math). 4-way PE column tiling (tile_position col groups) packs 4
batch-32 matmuls concurrently, writing z in a stacked PSUM layout:
  z_ps [128, 1024]: position (32j+b, 512*beta + n) = z_perm[2048*beta + 512j + n]
Weight columns are host-permuted so that
  bank0 (cols 0:512)  = [ sig-gate i | sig-gate f ] column-paired per channel
  bank1 (cols 512:1024) = [ tanh-gate g | sig-gate o ]
with channel(p=32j+b, n) = 256j + (n mod 256): all gate elementwise ops are
partition-aligned and the c update is a free-dim-shifted add.
h/y are fed back transposed via PE transpose-mode matmuls.

Overlap structure: the x-part matmuls of step t+1 are software-pipelined into
step t, z-chunks are ordered x->h->y so the y-feedback chain hides under the
h-matmul span, and the gate chain runs in two column halves. The pipeline
restarts at each loop back-edge (a full barrier); loop-carried state (h^T,
y^T, c) lives in fixed SBUF tiles written in place.

Measured (axon tunnel, cached-compile re-run incl. transfers): ~1.25s
end-to-end for the full call, vs ~8.1s for the session-start baseline
(fully unrolled bf16 kernel shipping replicated fp32-I/O tensors).
"""

import sys

for p in ("/opt/trn_rl_repo",):
    if p not in sys.path:
        sys.path.insert(0, p)

from contextlib import ExitStack

import numpy as np

import concourse.bacc as bacc
import concourse.bass as bass
import concourse.mybir as mybir
import concourse.tile as tile
from concourse.bass import ds
from concourse.masks import make_identity

F32 = mybir.dt.float32
U8 = mybir.dt.uint8
AF = mybir.ActivationFunctionType
ALU = mybir.AluOpType

B, T, D, H, O = 256, 256, 256, 1024, 256
NCORES = 8
BL = B // NCORES  # 32
G4 = 4 * H  # 4096
KX, KY, KH = D // 128, O // 128, H // 128  # 2, 2, 8
WXS = D // NCORES  # 32 weight-shard rows per core for Wxx/Wxy
WHS = H // NCORES  # 128 shard rows for Wh/Wd
# flat per-core weight shard: [Wxx | Wxy | Wh | Wd] row-shards, one tensor so
# the tunnel pays one per-array transfer latency instead of four
WOFF = (0, WXS * G4, 2 * WXS * G4, 2 * WXS * G4 + WHS * G4,
        2 * WXS * G4 + WHS * G4 + WHS * O)
WSH_N = WOFF[4]  # 819200
# x ships as XBITS-bit fixed point: u = round(x/s) + 2^(XBITS-1),
# s = max|x|/(2^(XBITS-1)-1), packed as byte planes (12-bit: 2 values ->
# 3 bytes; 10-bit: 4 values -> 5 bytes). The device unpacks to the integer
# v = u - 2^(XBITS-1) in fp16 (exact); s is folded into Wxx host-side.
X12 = True
XBITS = 10
XOFF = 1 << (XBITS - 1)
XQ = XOFF - 1
XW = {12: 3 * BL, 10: 5 * BL // 2}[XBITS]

# u8 output encoding: u = convert_u8(127*y + U8_BIAS), decoded (u-128)/127.
# The DVE float->u8 convert rounds-to-nearest (measured on hw: mean code
# offset -0.001, std 0.289), so the bias is exactly 128.0.
U8_BIAS_DEFAULT = 128.0


def gate_perm() -> np.ndarray:
    """Map stored z column position -> original gate column (i,f,g,o order)."""
    perm = np.empty(G4, dtype=np.int64)
    for beta in (0, 1):
        for j in range(4):
            for half in (0, 1):
                gate = (0, 1, 2, 3)[2 * beta + half]
                src = 1024 * gate + 256 * j
                pos = 2048 * beta + 512 * j + 256 * half
                perm[pos : pos + 256] = np.arange(src, src + 256)
    return perm


def _hT_off(c: int) -> int:
    """Column offset of h^T chunk c (channels 128c:128c+128) inside hT_sb."""
    return 128 * (c % 2) + 32 * (c // 2)


def build_nc(T_steps: int = T, use_bias_z: bool = False, use_bias_y: bool = False,
             mm_dt=mybir.dt.float16, out_u8: bool = True,
             u8_bias: float = U8_BIAS_DEFAULT):
    nc = bacc.Bacc()

    if X12:
        xT_d = nc.declare_dram_parameter("xT", [T_steps, 128, XW], U8,
                                         isOutput=False)
    else:
        xT_d = nc.declare_dram_parameter("xT", [T_steps, 128, 2 * BL], mm_dt,
                                         isOutput=False)
    # weights arrive as one flat row-shard: core c holds rows
    # [c*WXS:(c+1)*WXS] of Wxx/Wxy and rows [c*WHS:(c+1)*WHS] of Wh/Wd;
    # AllGather rebuilds the full matrices in DRAM (saves 7/8 of the weight
    # bytes over the tunnel).
    wsh_d = nc.declare_dram_parameter("wsh", [WSH_N], mm_dt, isOutput=False)
    bz_d = by_d = None
    if use_bias_z:
        bz_d = nc.declare_dram_parameter("bz", [128, 1024], F32, isOutput=False)
    if use_bias_y:
        by_d = nc.declare_dram_parameter("by", [BL, O], F32, isOutput=False)
    out_dt = U8 if out_u8 else mm_dt
    # t-major so the per-step store is one outer-dim (dynamic) slice
    ys_d = nc.declare_dram_parameter("ys", [T_steps, BL, O], out_dt,
                                     isOutput=True)

    def mc(ap):
        return ap.bitcast(mm_dt) if ap.dtype != mm_dt else ap

    with tile.TileContext(nc) as tc:
        with ExitStack() as ctx:
            dpool = ctx.enter_context(
                tc.tile_pool(name="dramw", bufs=1, space="DRAM"))
            wpool = ctx.enter_context(tc.tile_pool(name="weights", bufs=1))
            state = ctx.enter_context(tc.tile_pool(name="state", bufs=1))
            xpool = ctx.enter_context(tc.tile_pool(name="xin", bufs=3))
            gpool = ctx.enter_context(tc.tile_pool(name="gates", bufs=1))
            hpool = ctx.enter_context(tc.tile_pool(name="hT", bufs=1))
            ypool = ctx.enter_context(tc.tile_pool(name="yt", bufs=1))
            zpsum = ctx.enter_context(tc.tile_pool(name="zps", bufs=2, space="PSUM"))
            ypsum = ctx.enter_context(tc.tile_pool(name="yps", bufs=2, space="PSUM"))
            tpsum = ctx.enter_context(tc.tile_pool(name="tps", bufs=2, space="PSUM"))

            # c state, channel(32j+b, n) = 256j + n  (memset first: needed at
            # t=0, and it shares the gpsimd queue with the gathers below)
            c_sb = state.tile([128, 256], F32)
            nc.gpsimd.memset(c_sb[:], 0.0)

            # ---- on-device weight reassembly: DRAM AllGather per matrix ----
            # bounce buffer: collectives can't operate on I/O tensors. One
            # bounce DMA, then four gathers reading flat slices of it; each
            # gather's output lands in the matrix's natural row-major layout
            # (rank r's rows land at block r).
            wsh_b = dpool.tile([WSH_N], mm_dt, name="wsh_b")
            nc.gpsimd.dma_start(wsh_b[:], wsh_d[:])
            Wxx_g = dpool.tile([D, G4], mm_dt, name="Wxx_g")
            Wxy_g = dpool.tile([O, G4], mm_dt, name="Wxy_g")
            Wh_g = dpool.tile([H, G4], mm_dt, name="Wh_g")
            Wd_g = dpool.tile([H, O], mm_dt, name="Wd_g")
            RG = [list(range(NCORES))]
            # gather order = first-use order: Wxx (t=0 z), Wd (t=0 y), Wh/Wxy
            # (t=1 z)
            for (a, b), g in (((WOFF[0], WOFF[1]), Wxx_g),
                              ((WOFF[3], WOFF[4]), Wd_g),
                              ((WOFF[2], WOFF[3]), Wh_g),
                              ((WOFF[1], WOFF[2]), Wxy_g)):
                nc.gpsimd.collective_compute(
                    "AllGather", ALU.bypass, replica_groups=RG,
                    ins=[wsh_b[a:b].opt()], outs=[g.opt()])

            Wxx_sb = wpool.tile([128, KX * G4], mm_dt)
            Wxy_sb = wpool.tile([128, KY * G4], mm_dt)
            Wh_sb = wpool.tile([128, KH * G4], mm_dt)
            Wd_sb = wpool.tile([128, KH * O], mm_dt)
            # Matmult instructions can carry at most ONE sem wait in this
            # lowering; every matmul dependency must resolve to a single DVE
            # sem value. Weight DMAs are therefore "laundered" through
            # in-place DVE copies (one per DMA so each copy waits on one
            # DMA-queue sem only).
            for k in range(KX):
                nc.sync.dma_start(Wxx_sb[:, k * G4 : (k + 1) * G4],
                                  Wxx_g[k * 128 : (k + 1) * 128, :])
                nc.vector.tensor_copy(Wxx_sb[:, k * G4 : (k + 1) * G4],
                                      Wxx_sb[:, k * G4 : (k + 1) * G4])
            for k in range(KY):
                nc.sync.dma_start(Wxy_sb[:, k * G4 : (k + 1) * G4],
                                  Wxy_g[k * 128 : (k + 1) * 128, :])
                nc.vector.tensor_copy(Wxy_sb[:, k * G4 : (k + 1) * G4],
                                      Wxy_sb[:, k * G4 : (k + 1) * G4])
            for k in range(KH):
                nc.sync.dma_start(Wh_sb[:, k * G4 : (k + 1) * G4],
                                  Wh_g[k * 128 : (k + 1) * 128, :])
                nc.vector.tensor_copy(Wh_sb[:, k * G4 : (k + 1) * G4],
                                      Wh_sb[:, k * G4 : (k + 1) * G4])
                nc.sync.dma_start(Wd_sb[:, k * O : (k + 1) * O],
                                  Wd_g[k * 128 : (k + 1) * 128, :])
                nc.vector.tensor_copy(Wd_sb[:, k * O : (k + 1) * O],
                                      Wd_sb[:, k * O : (k + 1) * O])
            if use_bias_z:
                bz_sb = wpool.tile([128, 1024], F32)
                nc.sync.dma_start(bz_sb[:], bz_d[:, :])
            if use_bias_y:
                by_sb = wpool.tile([BL, O], F32)
                nc.sync.dma_start(by_sb[:], by_d[:, :])

            # identity for PE transposes (16-bit: f32 transpose-mode faults on
            # hw); I64 in both partition halves so the fmap can start at
            # partition 0 or 64 (must match the weights)
            ident = wpool.tile([128, 128], mm_dt)
            make_identity(nc, ident[:])
            nc.vector.tensor_copy(ident[:], ident[:])  # launder Pool dep -> DVE

            def emit_z_mms(z_tile, chunks, start, stop):
                nck = len(chunks)
                for ci, (lhsT, wtile, coff) in enumerate(chunks):
                    for beta in range(2):
                        for j in range(4):
                            w_lo = coff + 2048 * beta + 512 * j
                            nc.tensor.matmul(
                                z_tile[32 * j : 32 * (j + 1),
                                       512 * beta : 512 * (beta + 1)],
                                mc(lhsT),
                                mc(wtile[:, w_lo : w_lo + 512]),
                                start=(start and ci == 0),
                                stop=(stop and ci == nck - 1),
                                tile_position=(0, 32 * j),
                                skip_group_check=True,
                            )

            def load_x(idx):
                """idx: python int or ScalarValue (dynamic) step index."""
                xw = XW if X12 else 2 * BL
                xT_sb = xpool.tile([128, xw], U8 if X12 else mm_dt,
                                   name="xT_sb")
                if isinstance(idx, int):
                    nc.sync.dma_start(xT_sb[:], xT_d[idx])
                else:
                    nc.sync.dma_start(xT_sb[:], xT_d[ds(idx, 1)].squeeze(0))
                xr_sb = xpool.tile([128, 2 * BL], mm_dt, name="xr_sb")
                if X12 and XBITS == 12:
                    # unpack byte planes B0|B1|B2 -> integer v = u - 2048 in
                    # fp16 (exact for |v| <= 2047); these DVE ops also launder
                    # the x DMA-queue sem into the DVE sem
                    B0 = xT_sb[:, 0:BL]
                    B1 = xT_sb[:, BL : 2 * BL]
                    B2 = xT_sb[:, 2 * BL : 3 * BL]
                    b1c = xpool.tile([128, BL], mybir.dt.uint16, name="b1c")
                    ahi = xpool.tile([128, BL], mybir.dt.uint16, name="ahi")
                    alo = xpool.tile([128, BL], mybir.dt.uint16, name="alo")
                    # bitwise ops can't cast, so widen B1 via copy first;
                    # fused (op0, op1) pairs must also be same ALU class
                    nc.vector.tensor_copy(b1c[:], B1)
                    nc.vector.tensor_scalar(ahi[:], b1c[:], 0x0F, None,
                                            ALU.bitwise_and)
                    nc.vector.tensor_scalar(ahi[:], ahi[:], 256, None,
                                            ALU.mult)
                    nc.vector.tensor_copy(alo[:], B0)
                    nc.vector.tensor_add(ahi[:], ahi[:], alo[:])
                    nc.vector.tensor_scalar(
                        xr_sb[:, 0 : 2 * BL : 2], ahi[:], 2048, None,
                        ALU.subtract)
                    nc.vector.tensor_scalar(b1c[:], b1c[:], 4, None,
                                            ALU.logical_shift_right)
                    nc.vector.tensor_scalar(alo[:], B2, 16, None, ALU.mult)
                    nc.vector.tensor_add(b1c[:], b1c[:], alo[:])
                    nc.vector.tensor_scalar(
                        xr_sb[:, 1 : 2 * BL : 2], b1c[:], 2048, None,
                        ALU.subtract)
                elif X12:
                    # 10-bit: planes P0..P4, quads u0..u3 per 5 bytes.
                    # u0 = P0 + ((P1 & 3) << 8);  u1 = (P1>>2) + ((P2&15)<<6)
                    # u2 = (P2>>4) + ((P3&63)<<4); u3 = (P3>>6) + (P4<<2)
                    # Bitwise ops can't cast (widen via copies first) and
                    # fuse only with bitwise; (mask,shift) pairs fuse.
                    G = BL // 2  # 16 plane columns
                    c = []
                    for i in range(5):
                        ci = xpool.tile([128, G], mybir.dt.uint16,
                                        name=f"xc{i}")
                        nc.vector.tensor_copy(ci[:], xT_sb[:, G * i : G * (i + 1)])
                        c.append(ci)
                    t = xpool.tile([128, G], mybir.dt.uint16, name="xt0")
                    s2 = xpool.tile([128, G], mybir.dt.uint16, name="xt1")
                    for q, (lo_src, lo_shr, hi_src, hi_mask, hi_shl) in (
                        (0, (c[0], 0, c[1], 0x03, 8)),
                        (1, (c[1], 2, c[2], 0x0F, 6)),
                        (2, (c[2], 4, c[3], 0x3F, 4)),
                        (3, (c[3], 6, c[4], None, 2)),
                    ):
                        if hi_mask is not None:
                            nc.vector.tensor_scalar(s2[:], hi_src[:], hi_mask,
                                                    hi_shl, ALU.bitwise_and,
                                                    ALU.logical_shift_left)
                        else:
                            nc.vector.tensor_scalar(s2[:], hi_src[:], hi_shl,
                                                    None,
                                                    ALU.logical_shift_left)
                        if lo_shr:
                            nc.vector.tensor_scalar(t[:], lo_src[:], lo_shr,
                                                    None,
                                                    ALU.logical_shift_right)
                            nc.vector.tensor_add(t[:], t[:], s2[:])
                        else:
                            nc.vector.tensor_add(t[:], lo_src[:], s2[:])
                        nc.vector.tensor_scalar(
                            xr_sb[:, q : 2 * BL : 4], t[:], XOFF, None,
                            ALU.subtract)
                else:
                    # launder the x DMA-queue sem into the DVE sem
                    nc.vector.tensor_copy(xr_sb[:], xT_sb[:])
                return [(xr_sb[:, bass.ts(k, BL)], Wxx_sb, k * G4)
                        for k in range(KX)]

            # Loop-carried feedback state must be FIXED tiles written in
            # place (like c_sb): per-iteration pool allocations read via a
            # pre-loop handle deadlock the tile scheduler at the back edge.
            # The uniform loop body always runs the h/y matmuls, so step 0
            # consumes the memset h_{-1}=y_{-1}=0 state.
            hT_state = hpool.tile([128, 256], mm_dt, name="hT_st")
            nc.gpsimd.memset(hT_state[:], 0.0)
            yT_state = ypool.tile([128, 2 * BL], mm_dt, name="yT_st")
            nc.gpsimd.memset(yT_state[:], 0.0)

            UNROLL = 8
            assert T_steps % UNROLL == 0

            # software pipeline: within a group, the x-part of step t+1 is
            # issued during step t, so the in-order PE has independent work
            # while the gate chain (ACT/DVE) of step t runs. The pipeline
            # restarts at each group boundary (the loop back-edge is a full
            # barrier), costing a few us per group.
            with tc.For_i(0, T_steps, UNROLL) as t0:
                z_ps = zpsum.tile([128, 1024], F32, name="z_ps")
                emit_z_mms(z_ps, load_x(t0), start=True, stop=False)
                for j in range(UNROLL):
                    # h first, y last: the y feedback chain (Wd+tanh+cast+
                    # transpose) of step t-1 gets the h-matmul span as slack
                    chunks = [(hT_state[:, _hT_off(k) : _hT_off(k) + BL],
                               Wh_sb, k * G4) for k in (0, 2, 4, 6, 1, 3, 5, 7)]
                    chunks += [(yT_state[:, bass.ts(k, BL)], Wxy_sb, k * G4)
                               for k in range(KY)]
                    emit_z_mms(z_ps, chunks, start=False, stop=True)
                    if j + 1 < UNROLL:
                        z_next = zpsum.tile([128, 1024], F32, name="z_ps")
                        emit_z_mms(z_next, load_x(t0 + (j + 1)), start=True,
                                   stop=False)
                    else:
                        z_next = None

                    # gate math: <=1 PSUM operand per DVE op
                    if use_bias_z:
                        nc.vector.tensor_add(z_ps[:, 0:512], z_ps[:, 0:512],
                                             bz_sb[:, 0:512])
                        nc.vector.tensor_add(z_ps[:, 512:1024],
                                             z_ps[:, 512:1024],
                                             bz_sb[:, 512:1024])
                    # gate chain split into column halves: half 0 finishes ->
                    # its transpose + hT copy run while half 1 still computes,
                    # so the even hT-chunk matmuls of step t+1 start earlier
                    tg_sb = gpool.tile([128, 256], F32, name="tg_sb")
                    o_sb = gpool.tile([128, 256], F32, name="o_sb")
                    h_stk = gpool.tile([128, 256], mm_dt, name="h_stk")
                    tr_ps = tpsum.tile([128, 320], mm_dt, name="tr_ps")
                    hT_sb = hT_state
                    for hf in range(2):
                        s = slice(128 * hf, 128 * hf + 128)
                        nc.scalar.activation(tg_sb[:, s],
                                             z_ps[:, 512 + 128 * hf :
                                                  640 + 128 * hf],
                                             AF.Tanh)
                        nc.scalar.activation(z_ps[:, s], z_ps[:, s], AF.Sigmoid)
                        nc.vector.tensor_mul(tg_sb[:, s], z_ps[:, s],
                                             tg_sb[:, s])
                        nc.scalar.activation(z_ps[:, 256 + 128 * hf :
                                                  384 + 128 * hf],
                                             z_ps[:, 256 + 128 * hf :
                                                  384 + 128 * hf],
                                             AF.Sigmoid)
                        nc.vector.tensor_mul(c_sb[:, s],
                                             z_ps[:, 256 + 128 * hf :
                                                  384 + 128 * hf],
                                             c_sb[:, s])
                        nc.scalar.activation(o_sb[:, s],
                                             z_ps[:, 768 + 128 * hf :
                                                  896 + 128 * hf],
                                             AF.Sigmoid)
                        nc.vector.tensor_add(c_sb[:, s], tg_sb[:, s],
                                             c_sb[:, s])
                        nc.scalar.activation(tg_sb[:, s], c_sb[:, s], AF.Tanh)
                        nc.vector.tensor_mul(h_stk[:, s], o_sb[:, s],
                                             tg_sb[:, s])
                        nc.tensor.transpose(tr_ps[:, s], h_stk[:, s], ident[:])
                        nc.vector.tensor_copy(hT_sb[:, s], tr_ps[:, s])

                    # y = tanh(h @ Wd + bd)
                    y_ps = ypsum.tile([BL, O], F32, name="y_ps")
                    for k in range(KH):
                        nc.tensor.matmul(
                            y_ps[:],
                            mc(hT_sb[:, _hT_off(k) : _hT_off(k) + BL]),
                            mc(Wd_sb[:, k * O : (k + 1) * O]),
                            start=(k == 0),
                            stop=(k == KH - 1),
                        )
                    if use_bias_y:
                        nc.vector.tensor_add(y_ps[:], y_ps[:], by_sb[:])
                    y_sb = ypool.tile([BL, O], F32, name="y_sb", bufs=2)
                    nc.scalar.activation(y_sb[:], y_ps[:], AF.Tanh)
                    # cast y for the fp16 PE-transposes (also launders
                    # ACT -> DVE); this is on the feedback critical path, so
                    # it runs before the u8 output quantization
                    y_bf = ypool.tile([BL, O], mm_dt, name="y_bf")
                    nc.vector.tensor_copy(y_bf[:], y_sb[:])
                    # own double-buffered tile so the output DMA never blocks
                    # the next step's gate ACTs
                    if out_u8:
                        y_out = ypool.tile([BL, O], U8, name="y_out", bufs=2)
                        nc.vector.tensor_scalar(y_out[:], y_sb[:], 127.0,
                                                u8_bias, ALU.mult, ALU.add)
                    else:
                        y_out = ypool.tile([BL, O], mm_dt, name="y_out",
                                           bufs=2)
                        nc.vector.tensor_copy(y_out[:], y_sb[:])
                    nc.sync.dma_start(ys_d[ds(t0 + j, 1)].squeeze(0), y_out[:])

                    # y -> yT via 2 PE transposes
                    for q in range(2):
                        nc.tensor.transpose(
                            tr_ps[:, 256 + 32 * q : 256 + 32 * (q + 1)],
                            y_bf[0:BL, 128 * q : 128 * (q + 1)],
                            ident[0:32, 0:32],
                        )
                    nc.vector.tensor_copy(yT_state[:], tr_ps[:, 256:320])

                    z_ps = z_next

    nc.compile()
    return nc


def prep_inputs(x, Wx, Wh, b, Wd, bd, T_steps: int = T,
                mm_np=np.float16):
    """Host-side shard + relayout. Returns (in_maps, use_bias_z, use_bias_y)."""
    x = np.asarray(x, dtype=np.float32)[:, :T_steps, :]
    Wx = np.asarray(Wx, dtype=np.float32)
    Wh = np.asarray(Wh, dtype=np.float32)
    b = np.asarray(b, dtype=np.float32)
    Wd = np.asarray(Wd, dtype=np.float32)
    bd = np.asarray(bd, dtype=np.float32)

    perm = gate_perm()
    if X12:
        xs = max(float(np.abs(x).max()), 1e-20) / XQ  # folded into Wxx
    else:
        xs = 1.0
    Wxp = Wx[:, perm]
    Wxx = np.ascontiguousarray(Wxp[:D] * xs).astype(mm_np)
    Wxy = np.ascontiguousarray(Wxp[D:]).astype(mm_np)
    Whp = np.ascontiguousarray(Wh[:, perm]).astype(mm_np)
    Wd = Wd.astype(mm_np)

    use_bias_z = bool(np.any(b))
    use_bias_y = bool(np.any(bd))
    shared = {}
    if use_bias_z:
        bp = b[perm]
        bz = np.empty((128, 1024), dtype=np.float32)
        for j in range(4):
            for beta in range(2):
                bz[32 * j : 32 * (j + 1), 512 * beta : 512 * (beta + 1)] = bp[
                    2048 * beta + 512 * j : 2048 * beta + 512 * j + 512][None, :]
        shared["bz"] = bz
    if use_bias_y:
        shared["by"] = np.broadcast_to(bd, (BL, O)).copy()

    if X12:
        xu = (np.round(x / xs).astype(np.int32) + XOFF).astype(np.uint16)
    in_maps = []
    for c in range(NCORES):
        if X12:
            xc = xu[c * BL : (c + 1) * BL]                 # [BL, T, D] u16
        else:
            xc = x[c * BL : (c + 1) * BL]
        xT = xc.transpose(1, 2, 0)                         # [T, D, BL]
        xT = xT.reshape(T_steps, 2, 128, BL).transpose(0, 2, 1, 3)
        xT = xT.reshape(T_steps, 128, 2 * BL)
        if X12 and XBITS == 12:
            a = xT[:, :, 0::2].astype(np.uint16)           # [T, 128, BL]
            bb = xT[:, :, 1::2].astype(np.uint16)
            B0 = (a & 0xFF).astype(np.uint8)
            B1 = ((a >> 8) | ((bb & 0xF) << 4)).astype(np.uint8)
            B2 = (bb >> 4).astype(np.uint8)
            xT = np.ascontiguousarray(
                np.concatenate([B0, B1, B2], axis=2))      # [T, 128, 3*BL]
        elif X12:
            u0 = xT[:, :, 0::4].astype(np.uint16)          # [T, 128, BL/2]
            u1 = xT[:, :, 1::4].astype(np.uint16)
            u2 = xT[:, :, 2::4].astype(np.uint16)
            u3 = xT[:, :, 3::4].astype(np.uint16)
            P0 = (u0 & 0xFF).astype(np.uint8)
            P1 = ((u0 >> 8) | ((u1 & 0x3F) << 2)).astype(np.uint8)
            P2 = ((u1 >> 6) | ((u2 & 0x0F) << 4)).astype(np.uint8)
            P3 = ((u2 >> 4) | ((u3 & 0x03) << 6)).astype(np.uint8)
            P4 = (u3 >> 2).astype(np.uint8)
            xT = np.ascontiguousarray(
                np.concatenate([P0, P1, P2, P3, P4], axis=2))  # [T,128,XW]
        else:
            xT = np.ascontiguousarray(xT).astype(mm_np)
        wsh = np.concatenate([
            Wxx[c * WXS : (c + 1) * WXS].ravel(),
            Wxy[c * WXS : (c + 1) * WXS].ravel(),
            Whp[c * WHS : (c + 1) * WHS].ravel(),
            Wd[c * WHS : (c + 1) * WHS].ravel(),
        ])
        in_maps.append({"xT": xT, "wsh": wsh, **shared})
    return in_maps, use_bias_z, use_bias_y


def decode_ys(res, out_u8: bool = True):
    """Concatenate per-core results and decode to fp32 [B, T, O]."""
    parts = []
    for c in range(NCORES):
        ys = res.results[c]["ys"]                 # [T, BL, O] t-major
        ys = np.ascontiguousarray(ys.transpose(1, 0, 2))  # [BL, T, O]
        if out_u8:
            parts.append((ys.astype(np.float32) - 128.0) * (1.0 / 127.0))
        else:
            parts.append(ys.astype(np.float32))
    return np.concatenate(parts, axis=0)


_NC_CACHE = {}


def kernel(x, Wx, Wh, b, Wd, bd):
    from concourse.bass_utils import run_bass_kernel_spmd

    in_maps, ubz, uby = prep_inputs(x, Wx, Wh, b, Wd, bd, T)
    key = (T, ubz, uby)
    nc = _NC_CACHE.get(key)
    if nc is None:
        nc = _NC_CACHE[key] = build_nc(T, ubz, uby)
    res = run_bass_kernel_spmd(nc, in_maps, list(range(NCORES)))
    return decode_ys(res)
